# revision 32
# baseline (speedup 1.0000x reference)
"""ChannelAttn Trainium2 kernel v3.1: bf16 hi/lo score path + packed value
path + chunked AllToAll tail.

Sharding: core c handles batch b=c//4, rows [2048*(c%4), 2048*(c%4+1)).
Replica groups per batch for score AllReduce: [[0,1,2,3],[4,5,6,7]].

Score path (softmax logits need ~fp32 accuracy; every single-precision
shortcut measurably breaks the 2e-2 gate):
  M = 1/2 xh^T xh + xh^T xl2 via bf16 hi/lo (xh=bf16(x), xl2=bf16(2(x-xh)),
  host-prepped), scaled 0.5 on the PSUM->SBUF copy; G = M + M^T locally
  (fp32 PE transposes). A = G Wq^T in THREE bf16 passes using hi/lo splits
  of both G (on-chip DVE split) and Wq^T (host): gh*wh + gh*wl + gl*wh
  (error ~5e-4 in logits). Scores S^T_h = wk-contract A in true fp32.
  AllReduce partial scores over the batch group (hidden behind phase V);
  softmax over the free dim; PE-transpose pieces into a block-diagonal
  SM [768x768, packed (h,d) x (h,e)] bf16 (zero-filled by gpsimd memset).

Value path: vT[(h,d) packed, n] = Wv^T-contract xhT where xhT is shipped
pre-transposed bf16 from the host (no on-chip transpose phase);
x_caT = SM^T-contract vT, 14 matmuls per 512-wide window (block-diag
skips zero blocks); AllToAll in 4 e-chunks so each chunk's projection
tiles (z transpose + Wp matmul + bias) pipeline behind later chunks.

Shapes hardcoded: B=2, N=8192, C=768, H=4, HD=192.
"""

import sys

sys.path.insert(0, "/opt/trn_rl_repo")

import numpy as np

B, N, C, H = 2, 8192, 768, 4
HD = C // H  # 192
NCORE = 8
ROWS = N // 4  # 2048 rows per core (of one batch)
NCH = ROWS // 128  # 16 chunks
NW = ROWS // 512  # 4 windows
NCHUNK = 4  # a2a chunks (6 e-columns of each dst's 24 per chunk)
ECH = 24 // NCHUNK  # e-cols per dst per chunk

_cached = {}


def _build():
    import concourse.bacc as bacc
    import concourse.mybir as mybir
    import concourse.tile as tile

    f32 = mybir.dt.float32
    bf16 = mybir.dt.bfloat16

    nc = bacc.Bacc("TRN2", target_bir_lowering=False, debug=False)

    xh_d = nc.dram_tensor("xh", [ROWS, C], bf16, kind="ExternalInput")
    xl2_d = nc.dram_tensor("xl2", [ROWS, C], bf16, kind="ExternalInput")
    xht_d = nc.dram_tensor("xht", [C, ROWS], bf16, kind="ExternalInput")
    wqth_d = nc.dram_tensor("wqth", [C, C], bf16, kind="ExternalInput")
    wqtl_d = nc.dram_tensor("wqtl", [C, C], bf16, kind="ExternalInput")
    wkt_d = nc.dram_tensor("wkt", [C, C], f32, kind="ExternalInput")
    wvt_d = nc.dram_tensor("wvt", [C, C], bf16, kind="ExternalInput")
    wpt_d = nc.dram_tensor("wpt", [C, C], bf16, kind="ExternalInput")
    bias_d = nc.dram_tensor("bias", [128, C], f32, kind="ExternalInput")
    tvec_d = nc.dram_tensor("tvec", [128, 4], f32, kind="ExternalInput")
    identb_d = nc.dram_tensor("identb", [128, 128], bf16, kind="ExternalInput")
    identf_d = nc.dram_tensor("identf", [128, 128], f32, kind="ExternalInput")
    out_d = nc.dram_tensor("out", [2, 1024, C], bf16, kind="ExternalOutput")

    RG = [[0, 1, 2, 3], [4, 5, 6, 7]]

    with tile.TileContext(nc) as tc:
        with (
            tc.tile_pool(name="wpool", bufs=1) as wpool,
            tc.tile_pool(name="drpool", bufs=1, space="DRAM") as drpool,
        ):
            # ---- long-lived pools (stack order = reverse close order) ----
            sm_cm = tc.tile_pool(name="smpool", bufs=1)
            smpool = sm_cm.__enter__()
            xht_cm = tc.tile_pool(name="xhtpool", bufs=1)
            xhtpool = xht_cm.__enter__()
            sc_cm = tc.tile_pool(name="scpool", bufs=1)
            scpool = sc_cm.__enter__()
            g_cm = tc.tile_pool(name="gpool", bufs=1)
            gpool = g_cm.__enter__()
            xio_cm = tc.tile_pool(name="xio", bufs=1)
            xio = xio_cm.__enter__()

            # ---- x hi/lo DMAs issued FIRST so phase G starts early ----
            xh_s = xio.tile([128, NCH, C], bf16, name="xh_s")
            xl2_s = xio.tile([128, NCH, C], bf16, name="xl2_s")
            for g in range(4):
                sl = slice(512 * g, 512 * (g + 1))
                eng_a = nc.sync if g % 2 == 0 else nc.scalar
                eng_b = nc.scalar if g % 2 == 0 else nc.sync
                eng_a.dma_start(
                    xh_s[:, 4 * g : 4 * (g + 1), :],
                    xh_d[sl, :].rearrange("(o p) f -> p o f", p=128),
                )
                eng_b.dma_start(
                    xl2_s[:, 4 * g : 4 * (g + 1), :],
                    xl2_d[sl, :].rearrange("(o p) f -> p o f", p=128),
                )
            xht_s = xhtpool.tile([128, 6, ROWS], bf16, name="xht_s")
            nc.scalar.dma_start(
                xht_s[:], xht_d.ap().rearrange("(o p) f -> p o f", p=128)
            )

            # ---- weights / constants (whole-kernel) ----
            wvt_s = wpool.tile([128, 6, C], bf16)
            nc.sync.dma_start(
                wvt_s[:], wvt_d.ap().rearrange("(o p) f -> p o f", p=128)
            )
            wpt_s = wpool.tile([128, 6, C], bf16)
            nc.sync.dma_start(
                wpt_s[:], wpt_d.ap().rearrange("(o p) f -> p o f", p=128)
            )
            bias_s = wpool.tile([128, C], f32)
            nc.sync.dma_start(bias_s[:], bias_d[:])
            tvec_s = wpool.tile([128, 4], f32)
            nc.sync.dma_start(tvec_s[:], tvec_d[:])
            identb = wpool.tile([128, 128], bf16)
            nc.sync.dma_start(identb[:], identb_d[:])
            identf = wpool.tile([128, 128], f32)
            nc.sync.dma_start(identf[:], identf_d[:])
            wqth_s = wpool.tile([128, 6, C], bf16, name="wqth_s")
            nc.scalar.dma_start(
                wqth_s[:], wqth_d.ap().rearrange("(o p) f -> p o f", p=128)
            )
            wqtl_s = wpool.tile([128, 6, C], bf16, name="wqtl_s")
            nc.scalar.dma_start(
                wqtl_s[:], wqtl_d.ap().rearrange("(o p) f -> p o f", p=128)
            )

            # DRAM staging
            ssend = drpool.tile([4, HD, HD], f32)
            srecv = drpool.tile([4, HD, HD], f32)
            a2asend = [
                drpool.tile([8, 4, ECH, ROWS], bf16, name=f"a2as{g}")
                for g in range(NCHUNK)
            ]
            a2arecv = [
                drpool.tile([8, 4, ECH, ROWS], bf16, name=f"a2ar{g}")
                for g in range(NCHUNK)
            ]
            vband = [
                drpool.tile([2, 4 * ECH, N], bf16, name=f"vband{g}")
                for g in range(NCHUNK)
            ]

            # block-diag softmax matrix, zeroed early on idle gpsimd
            sm_s = smpool.tile([128, 6, C], bf16, name="sm_s")
            nc.gpsimd.memset(sm_s[:], 0)

            # g-pool: g_f (fp32 G), g_bh/g_bl (bf16 hi/lo split)
            g_f = gpool.tile([128, 6, C], f32, name="g_f")
            g_bh = gpool.tile([128, 6, C], bf16, name="g_bh")
            g_bl = gpool.tile([128, 6, C], bf16, name="g_bl")

            msb_cm = tc.tile_pool(name="msbpool", bufs=1)
            msbpool = msb_cm.__enter__()
            msb = msbpool.tile([128, 6, C], f32, name="msb")

            # ============ Phase G: M = 1/2 xh^T xh + xh^T xl2 ==============
            with tc.tile_pool(name="ps_g", bufs=1, space="PSUM") as ps_g:
                for half in range(2):
                    csl = slice(384 * half, 384 * (half + 1))
                    mps = [
                        ps_g.tile([128, 384], f32, tag=f"m{j}", name=f"mps{j}")
                        for j in range(6)
                    ]
                    for ch in range(NCH):
                        for j in range(6):
                            lhs = xh_s[:, ch, 128 * j : 128 * (j + 1)]
                            nc.tensor.matmul(
                                mps[j][:], lhs, xh_s[:, ch, csl],
                                start=(ch == 0), stop=False,
                                skip_group_check=True,
                            )
                            nc.tensor.matmul(
                                mps[j][:], lhs, xl2_s[:, ch, csl],
                                start=False, stop=(ch == NCH - 1),
                                skip_group_check=True,
                            )
                    for j in range(6):
                        if j % 2 == 0:
                            nc.vector.tensor_scalar_mul(
                                msb[:, j, csl], mps[j][:], 0.5
                            )
                        else:
                            nc.scalar.mul(msb[:, j, csl], mps[j][:], 0.5)

            # ---- G = M + M^T, then bf16 hi/lo split of G ----
            with tc.tile_pool(name="ps_sym", bufs=2, space="PSUM") as ps_sym:
                for i in range(6):
                    trow = ps_sym.tile([128, C], f32, tag="trow")
                    for j in range(6):
                        nc.tensor.matmul(
                            trow[:, 128 * j : 128 * (j + 1)],
                            msb[:, j, 128 * i : 128 * (i + 1)],
                            identf[:], is_transpose=True,
                            start=True, stop=True, skip_group_check=True,
                        )
                    nc.vector.tensor_add(g_f[:, i, :], msb[:, i, :], trow[:])
                    nc.scalar.copy(g_bh[:, i, :], g_f[:, i, :])
                    nc.vector.tensor_sub(g_bl[:, i, :], g_f[:, i, :], g_bh[:, i, :])

            msb_cm.__exit__(None, None, None)  # free msb (18KB)
            xio_cm.__exit__(None, None, None)  # free xh/xl2 (49KB)

            # ============ Phase A: A = G Wq^T, 3 bf16 hi/lo passes =========
            wka_cm = tc.tile_pool(name="wkapool", bufs=1)
            wkapool = wka_cm.__enter__()
            wkt_s = wkapool.tile([128, 6, C], f32, name="wkt_s")
            nc.scalar.dma_start(
                wkt_s[:], wkt_d.ap().rearrange("(o p) f -> p o f", p=128)
            )
            a_s = wkapool.tile([128, 6, C], f32, name="a_s")
            APASS = [("hh", None, None), ("hl", None, None), ("lh", None, None)]
            with tc.tile_pool(name="ps_a", bufs=1, space="PSUM") as ps_a:
                for half in range(2):
                    qsl = slice(384 * half, 384 * (half + 1))
                    for i in range(6):
                        ap_t = ps_a.tile(
                            [128, 384], f32, tag=f"a{i}", name=f"aps{i}"
                        )
                        nmm = 0
                        for gt, wt in ((g_bh, wqth_s), (g_bh, wqtl_s), (g_bl, wqth_s)):
                            for j in range(6):
                                nmm += 1
                                nc.tensor.matmul(
                                    ap_t[:],
                                    gt[:, j, 128 * i : 128 * (i + 1)],
                                    wt[:, j, qsl],
                                    start=(nmm == 1),
                                    stop=(nmm == 18),
                                    skip_group_check=True,
                                )
                        if i % 2 == 0:
                            nc.vector.tensor_copy(a_s[:, i, qsl], ap_t[:])
                        else:
                            nc.scalar.copy(a_s[:, i, qsl], ap_t[:])

            # ---- scores S^T_h = wk-contract A (fp32), then AllReduce ----
            sp_lo = scpool.tile([128, 4, HD], f32)
            sp_hi = scpool.tile([64, 4, HD], f32)
            with tc.tile_pool(name="ps_sc", bufs=2, space="PSUM") as ps_sc:
                for h in range(4):
                    hsl = slice(HD * h, HD * (h + 1))
                    st_lo = ps_sc.tile([128, HD], f32, tag="stlo", name="st_lo")
                    st_hi = ps_sc.tile([64, HD], f32, tag="sthi", name="st_hi")
                    for i in range(6):
                        nc.tensor.matmul(
                            st_lo[:],
                            wkt_s[:, i, HD * h : HD * h + 128],
                            a_s[:, i, hsl],
                            start=(i == 0), stop=(i == 5),
                            skip_group_check=True,
                        )
                    for i in range(6):
                        nc.tensor.matmul(
                            st_hi[:],
                            wkt_s[:, i, HD * h + 128 : HD * (h + 1)],
                            a_s[:, i, hsl],
                            start=(i == 0), stop=(i == 5),
                            skip_group_check=True,
                        )
                    nc.vector.tensor_copy(sp_lo[:, h, :], st_lo[:])
                    nc.scalar.copy(sp_hi[:, h, :], st_hi[:])
            for h in range(4):
                nc.sync.dma_start(ssend[h, 0:128, :], sp_lo[:, h, :])
                nc.sync.dma_start(ssend[h, 128:HD, :], sp_hi[:, h, :])
            nc.gpsimd.collective_compute(
                "AllReduce",
                mybir.AluOpType.add,
                replica_groups=RG,
                ins=[ssend.opt()],
                outs=[srecv.opt()],
            )
            wka_cm.__exit__(None, None, None)  # free wkt_s + a_s (36KB)
            g_cm.__exit__(None, None, None)  # free g tiles (36KB)

            # ============ Phase V (packed, covers score-AllReduce) ==========
            vt_cm = tc.tile_pool(name="vtpool", bufs=1)
            vtpool = vt_cm.__enter__()
            vt_s = vtpool.tile([128, 6, ROWS], bf16, name="vt_s")
            with tc.tile_pool(name="ps_v", bufs=1, space="PSUM") as ps_v:
                for w in range(NW):
                    nsl = slice(512 * w, 512 * (w + 1))
                    for k in range(6):
                        vp = ps_v.tile(
                            [128, 512], f32, tag=f"v{k}", name=f"vps{k}"
                        )
                        for cb in range(6):
                            nc.tensor.matmul(
                                vp[:],
                                wvt_s[:, cb, 128 * k : 128 * (k + 1)],
                                xht_s[:, cb, nsl],
                                start=(cb == 0), stop=(cb == 5),
                            )
                        if k % 2 == 0:
                            nc.vector.tensor_copy(vt_s[:, k, nsl], vp[:])
                        else:
                            nc.scalar.copy(vt_s[:, k, nsl], vp[:])

            # ---- softmax per head from reduced scores -> block-diag SM ----
            sr_lo, sr_hi = sp_lo, sp_hi
            for h in range(4):
                nc.sync.dma_start(sr_lo[:, h, :], srecv[h, 0:128, :])
                nc.sync.dma_start(sr_hi[:, h, :], srecv[h, 128:HD, :])
            for h in range(4):
                smt = {}
                for src_t, nrow in ((sr_lo, 128), (sr_hi, 64)):
                    ap_in = src_t[0:nrow, h, :]
                    mx = scpool.tile([nrow, 1], f32, tag=f"mx{nrow}", name="mx")
                    nc.vector.tensor_reduce(
                        mx[:], ap_in,
                        axis=mybir.AxisListType.X, op=mybir.AluOpType.max,
                    )
                    nmt = scpool.tile([nrow, 1], f32, tag=f"nm{nrow}", name="nmt")
                    nc.vector.tensor_mul(nmt[:], mx[:], tvec_s[:nrow, h : h + 1])
                    nc.vector.tensor_scalar_mul(nmt[:], nmt[:], -1.0)
                    p_t = scpool.tile([nrow, HD], f32, tag=f"p{nrow}", name="p_t")
                    ssum = scpool.tile([nrow, 1], f32, tag=f"s{nrow}", name="ssum")
                    nc.scalar.activation(
                        p_t[:], ap_in,
                        mybir.ActivationFunctionType.Exp,
                        bias=nmt[:], scale=tvec_s[:nrow, h : h + 1],
                        accum_out=ssum[:],
                    )
                    rcp = scpool.tile([nrow, 1], f32, tag=f"r{nrow}", name="rcp")
                    nc.vector.reciprocal(rcp[:], ssum[:])
                    smt_t = scpool.tile(
                        [nrow, HD], f32, tag=f"smt{nrow}", name="smt_t"
                    )
                    nc.vector.tensor_scalar_mul(smt_t[:], p_t[:], rcp[:])
                    smt[nrow] = smt_t
                with tc.tile_pool(name=f"ps_smt{h}", bufs=1, space="PSUM") as pst:
                    tlo = pst.tile([128, HD], f32, name="tlo")
                    thi = pst.tile([64, HD], f32, name="thi")
                    nc.tensor.matmul(
                        tlo[:, 0:128], smt[128][:, 0:128], identf[:],
                        is_transpose=True, start=True, stop=True,
                        skip_group_check=True,
                    )
                    nc.tensor.matmul(
                        tlo[:, 128:HD], smt[64][:, 0:128], identf[:64, 0:64],
                        is_transpose=True, start=True, stop=True,
                        skip_group_check=True,
                    )
                    nc.tensor.matmul(
                        thi[:, 0:128], smt[128][:, 128:HD], identf[:],
                        is_transpose=True, start=True, stop=True,
                        skip_group_check=True,
                    )
                    nc.tensor.matmul(
                        thi[:, 128:HD], smt[64][:, 128:HD], identf[:64, 0:64],
                        is_transpose=True, start=True, stop=True,
                        skip_group_check=True,
                    )
                    # scatter into packed block-diag rows 192h + a
                    csl = slice(HD * h, HD * (h + 1))
                    for (src, a0, nr) in ((tlo, 0, 128), (thi, 128, 64)):
                        r0 = HD * h + a0
                        placed = 0
                        while placed < nr:
                            j = (r0 + placed) // 128
                            p0 = (r0 + placed) % 128
                            cnt = min(128 - p0, nr - placed)
                            nc.vector.tensor_copy(
                                sm_s[p0 : p0 + cnt, j, csl],
                                src[placed : placed + cnt, :],
                            )
                            placed += cnt

            # ============ Phase X: x_caT = SM^T-contract vT (packed) ========
            XJ = {0: (0, 1), 1: (0, 1, 2), 2: (1, 2), 3: (3, 4), 4: (3, 4, 5), 5: (4, 5)}
            xc_cm = tc.tile_pool(name="xcpool", bufs=1)
            xcpool = xc_cm.__enter__()
            xcat_s = xcpool.tile([128, 6, ROWS], bf16, name="xcat_s")
            # strip pieces grouped by the xcat tile j they read, so each
            # tile's a2a send staging fires as soon as that tile is done
            strip_by_tile = {k: [] for k in range(6)}
            for g in range(NCHUNK):
                for i in range(8):
                    for h in range(4):
                        r0 = 192 * h + 24 * i + ECH * g
                        placed = 0
                        while placed < ECH:
                            j = (r0 + placed) // 128
                            p0 = (r0 + placed) % 128
                            cnt = min(128 - p0, ECH - placed)
                            strip_by_tile[j].append((g, i, h, placed, p0, cnt))
                            placed += cnt
            with tc.tile_pool(name="ps_x", bufs=1, space="PSUM") as ps_x:
                for k in range(6):
                    for w in range(NW):
                        nsl = slice(512 * w, 512 * (w + 1))
                        xp = ps_x.tile(
                            [128, 512], f32, tag=f"x{w}", name=f"xps{w}"
                        )
                        js = XJ[k]
                        for idx, j in enumerate(js):
                            nc.tensor.matmul(
                                xp[:],
                                sm_s[:, j, 128 * k : 128 * (k + 1)],
                                vt_s[:, j, nsl],
                                start=(idx == 0),
                                stop=(idx == len(js) - 1),
                            )
                        if w % 2 == 0:
                            nc.vector.tensor_copy(xcat_s[:, k, nsl], xp[:])
                        else:
                            nc.scalar.copy(xcat_s[:, k, nsl], xp[:])
                    for (g, i, h, placed, p0, cnt) in strip_by_tile[k]:
                        if g == 0:
                            nc.sync.dma_start(
                                a2asend[g][i, h, placed : placed + cnt, :],
                                xcat_s[p0 : p0 + cnt, k, :],
                            )
                # remaining chunks' strips after all tiles, in chunk order,
                # so chunk g's send buffer completes ~one a2a ahead of use
                if k == 5:
                    for g in range(1, NCHUNK):
                        for kk in range(6):
                            for (gg, i, h, placed, p0, cnt) in strip_by_tile[kk]:
                                if gg == g:
                                    nc.sync.dma_start(
                                        a2asend[g][i, h, placed : placed + cnt, :],
                                        xcat_s[p0 : p0 + cnt, kk, :],
                                    )

            # ============ Tail: chunked a2a + projection pipeline ===========
            # chunk g: e-cols [24i+6g, 24i+6g+6) per dst i, all heads
            with (
                tc.tile_pool(name="zpool", bufs=3) as zpool,
                tc.tile_pool(name="opool", bufs=2) as opool,
                tc.tile_pool(name="ps_zt", bufs=2, space="PSUM") as ps_zt,
                tc.tile_pool(name="ps_o", bufs=2, space="PSUM") as ps_o,
            ):
                for g in range(NCHUNK):
                    nc.gpsimd.collective_compute(
                        "AllToAll",
                        mybir.AluOpType.bypass,
                        replica_groups=[list(range(8))],
                        ins=[a2asend[g].opt()],
                        outs=[a2arecv[g].opt()],
                    )
                    # assemble vband chunk: rows 4e+h, n from 4 senders
                    for b in range(2):
                        vb_v = vband[g][b].rearrange("(e h) n -> h e n", h=4)
                        for j in range(4):
                            eng = nc.sync if b == 0 else nc.scalar
                            eng.dma_start(
                                vb_v[:, :, ROWS * j : ROWS * (j + 1)],
                                a2arecv[g][4 * b + j],
                            )
                    # projection tiles: 2 per batch per chunk
                    for b in range(2):
                        for tt in range(2):
                            t = 2 * g + tt
                            z_nat = zpool.tile([128, C], bf16, tag="zn")
                            zeng = nc.scalar if b == 0 else nc.sync
                            zeng.dma_start(
                                z_nat[:],
                                vband[g][b, 12 * tt : 12 * (tt + 1), :],
                            )
                            ztp = ps_zt.tile([128, C], bf16, tag="ztp")
                            for j in range(6):
                                nc.tensor.matmul(
                                    ztp[:, 128 * j : 128 * (j + 1)],
                                    z_nat[:, 128 * j : 128 * (j + 1)],
                                    identb[:],
                                    is_transpose=True,
                                    start=True, stop=True,
                                    skip_group_check=True,
                                )
                            zt = zpool.tile([128, 6, 128], bf16, tag="zt")
                            if (b + tt) % 2 == 0:
                                nc.vector.tensor_copy(
                                    zt[:],
                                    ztp[:].rearrange("p (o f) -> p o f", f=128),
                                )
                            else:
                                nc.scalar.copy(
                                    zt[:],
                                    ztp[:].rearrange("p (o f) -> p o f", f=128),
                                )
                            o1 = ps_o.tile([128, 384], f32, tag="o1")
                            o2 = ps_o.tile([128, 384], f32, tag="o2")
                            for j in range(6):
                                nc.tensor.matmul(
                                    o1[:], zt[:, j, :], wpt_s[:, j, 0:384],
                                    start=(j == 0), stop=(j == 5),
                                )
                            for j in range(6):
                                nc.tensor.matmul(
                                    o2[:], zt[:, j, :], wpt_s[:, j, 384:C],
                                    start=(j == 0), stop=(j == 5),
                                )
                            out_sb = opool.tile([128, C], bf16, tag="ob")
                            nc.vector.tensor_add(
                                out_sb[:, 0:384], o1[:], bias_s[:, 0:384]
                            )
                            nc.vector.tensor_add(
                                out_sb[:, 384:C], o2[:], bias_s[:, 384:C]
                            )
                            oeng = nc.sync if b == 0 else nc.scalar
                            oeng.dma_start(
                                out_d[b, 128 * t : 128 * (t + 1), :], out_sb[:]
                            )

            xc_cm.__exit__(None, None, None)
            vt_cm.__exit__(None, None, None)
            sc_cm.__exit__(None, None, None)
            xht_cm.__exit__(None, None, None)
            sm_cm.__exit__(None, None, None)

    nc.compile()
    return nc


def _get_nc():
    if "nc" not in _cached:
        _cached["nc"] = _build()
    return _cached["nc"]


def _prep_in_maps(x, w_qkv, temperature, w_proj, b_proj):
    import ml_dtypes

    bf = ml_dtypes.bfloat16
    x = np.ascontiguousarray(np.asarray(x, dtype=np.float32))
    w_qkv = np.asarray(w_qkv, dtype=np.float32)
    temperature = np.asarray(temperature, dtype=np.float32)
    w_proj = np.asarray(w_proj, dtype=np.float32)
    b_proj = np.asarray(b_proj, dtype=np.float32)

    wqt = np.ascontiguousarray(w_qkv[0:C].T)
    wqth = wqt.astype(bf)
    wqtl = (wqt - wqth.astype(np.float32)).astype(bf)
    wkt = np.ascontiguousarray(w_qkv[C : 2 * C].T)
    wvt = np.ascontiguousarray(w_qkv[2 * C : 3 * C].T).astype(bf)
    wpt = np.ascontiguousarray(w_proj.T).astype(bf)

    bias = np.ascontiguousarray(np.broadcast_to(b_proj, (128, C)))
    tvec = np.broadcast_to(
        temperature.reshape(1, H).astype(np.float32), (128, H)
    ).copy()
    identb = np.eye(128, dtype=np.float32).astype(bf)
    identf = np.eye(128, dtype=np.float32)

    in_maps = []
    for c in range(NCORE):
        b, r = c // 4, c % 4
        xs = x[b, ROWS * r : ROWS * (r + 1), :]
        xh = xs.astype(bf)
        xl2 = ((xs - xh.astype(np.float32)) * 2.0).astype(bf)
        in_maps.append(
            {
                "xh": np.ascontiguousarray(xh),
                "xl2": np.ascontiguousarray(xl2),
                "xht": np.ascontiguousarray(xs.T).astype(bf),
                "wqth": wqth,
                "wqtl": wqtl,
                "wkt": wkt,
                "wvt": wvt,
                "wpt": wpt,
                "bias": bias,
                "tvec": tvec,
                "identb": identb,
                "identf": identf,
            }
        )
    return in_maps


def kernel(x, w_qkv, temperature, w_proj, b_proj):
    from concourse.bass_utils import run_bass_kernel_spmd

    nc = _get_nc()
    in_maps = _prep_in_maps(x, w_qkv, temperature, w_proj, b_proj)
    res = run_bass_kernel_spmd(nc, in_maps, core_ids=list(range(NCORE)))
    out = np.empty((B, N, C), np.float32)
    for c in range(NCORE):
        o = res.results[c]["out"]  # [2, 1024, C] bf16
        for b in range(B):
            out[b, 1024 * c : 1024 * (c + 1), :] = o[b].astype(np.float32)
    return out


# revision 33
# speedup vs baseline: 1.2212x; 1.2212x over previous
"""ChannelAttn Trainium2 kernel v3.1: bf16 hi/lo score path + packed value
path + chunked AllToAll tail.

Sharding: core c handles batch b=c//4, rows [2048*(c%4), 2048*(c%4+1)).
Replica groups per batch for score AllReduce: [[0,1,2,3],[4,5,6,7]].

Score path (softmax logits need ~fp32 accuracy; every single-precision
shortcut measurably breaks the 2e-2 gate):
  M = 1/2 xh^T xh + xh^T xl2 via bf16 hi/lo (xh=bf16(x), xl2=bf16(2(x-xh)),
  host-prepped), scaled 0.5 on the PSUM->SBUF copy; G = M + M^T locally
  (fp32 PE transposes). A = G Wq^T in THREE bf16 passes using hi/lo splits
  of both G (on-chip DVE split) and Wq^T (host): gh*wh + gh*wl + gl*wh
  (error ~5e-4 in logits). Scores S^T_h = wk-contract A in true fp32.
  AllReduce partial scores over the batch group (hidden behind phase V);
  softmax over the free dim; PE-transpose pieces into a block-diagonal
  SM [768x768, packed (h,d) x (h,e)] bf16 (zero-filled by gpsimd memset).

Value path: vT[(h,d) packed, n] = Wv^T-contract xhT where xhT is shipped
pre-transposed bf16 from the host (no on-chip transpose phase);
x_caT = SM^T-contract vT, 14 matmuls per 512-wide window (block-diag
skips zero blocks); AllToAll in 4 e-chunks so each chunk's projection
tiles (z transpose + Wp matmul + bias) pipeline behind later chunks.

Shapes hardcoded: B=2, N=8192, C=768, H=4, HD=192.
"""

import sys

sys.path.insert(0, "/opt/trn_rl_repo")

import numpy as np

B, N, C, H = 2, 8192, 768, 4
HD = C // H  # 192
NCORE = 8
ROWS = N // 4  # 2048 rows per core (of one batch)
NCH = ROWS // 128  # 16 chunks
NW = ROWS // 512  # 4 windows
NCHUNK = 2  # a2a chunks (e-columns of each dst's 24 split across chunks)
ECH = 24 // NCHUNK  # e-cols per dst per chunk

_cached = {}


def _build():
    import concourse.bacc as bacc
    import concourse.mybir as mybir
    import concourse.tile as tile

    f32 = mybir.dt.float32
    bf16 = mybir.dt.bfloat16

    nc = bacc.Bacc("TRN2", target_bir_lowering=False, debug=False)

    xh_d = nc.dram_tensor("xh", [ROWS, C], bf16, kind="ExternalInput")
    xl2_d = nc.dram_tensor("xl2", [ROWS, C], bf16, kind="ExternalInput")
    xht_d = nc.dram_tensor("xht", [C, ROWS], bf16, kind="ExternalInput")
    wqth_d = nc.dram_tensor("wqth", [C, C], bf16, kind="ExternalInput")
    wqtl_d = nc.dram_tensor("wqtl", [C, C], bf16, kind="ExternalInput")
    wkt_d = nc.dram_tensor("wkt", [C, C], f32, kind="ExternalInput")
    wvt_d = nc.dram_tensor("wvt", [C, C], bf16, kind="ExternalInput")
    wpt_d = nc.dram_tensor("wpt", [C, C], bf16, kind="ExternalInput")
    bias_d = nc.dram_tensor("bias", [128, C], f32, kind="ExternalInput")
    tvec_d = nc.dram_tensor("tvec", [128, 4], f32, kind="ExternalInput")
    identb_d = nc.dram_tensor("identb", [128, 128], bf16, kind="ExternalInput")
    identf_d = nc.dram_tensor("identf", [128, 128], f32, kind="ExternalInput")
    out_d = nc.dram_tensor("out", [2, 1024, C], bf16, kind="ExternalOutput")

    RG = [[0, 1, 2, 3], [4, 5, 6, 7]]

    with tile.TileContext(nc) as tc:
        with (
            tc.tile_pool(name="wpool", bufs=1) as wpool,
            tc.tile_pool(name="drpool", bufs=1, space="DRAM") as drpool,
        ):
            # ---- long-lived pools (stack order = reverse close order) ----
            sm_cm = tc.tile_pool(name="smpool", bufs=1)
            smpool = sm_cm.__enter__()
            xht_cm = tc.tile_pool(name="xhtpool", bufs=1)
            xhtpool = xht_cm.__enter__()
            sc_cm = tc.tile_pool(name="scpool", bufs=1)
            scpool = sc_cm.__enter__()
            g_cm = tc.tile_pool(name="gpool", bufs=1)
            gpool = g_cm.__enter__()
            xio_cm = tc.tile_pool(name="xio", bufs=1)
            xio = xio_cm.__enter__()

            # ---- x hi/lo DMAs issued FIRST so phase G starts early ----
            xh_s = xio.tile([128, NCH, C], bf16, name="xh_s")
            xl2_s = xio.tile([128, NCH, C], bf16, name="xl2_s")
            for g in range(4):
                sl = slice(512 * g, 512 * (g + 1))
                eng_a = nc.sync if g % 2 == 0 else nc.scalar
                eng_b = nc.scalar if g % 2 == 0 else nc.sync
                eng_a.dma_start(
                    xh_s[:, 4 * g : 4 * (g + 1), :],
                    xh_d[sl, :].rearrange("(o p) f -> p o f", p=128),
                )
                eng_b.dma_start(
                    xl2_s[:, 4 * g : 4 * (g + 1), :],
                    xl2_d[sl, :].rearrange("(o p) f -> p o f", p=128),
                )
            xht_s = xhtpool.tile([128, 6, ROWS], bf16, name="xht_s")
            nc.scalar.dma_start(
                xht_s[:], xht_d.ap().rearrange("(o p) f -> p o f", p=128)
            )

            # ---- weights / constants (whole-kernel) ----
            wvt_s = wpool.tile([128, 6, C], bf16)
            nc.sync.dma_start(
                wvt_s[:], wvt_d.ap().rearrange("(o p) f -> p o f", p=128)
            )
            wpt_s = wpool.tile([128, 6, C], bf16)
            nc.sync.dma_start(
                wpt_s[:], wpt_d.ap().rearrange("(o p) f -> p o f", p=128)
            )
            bias_s = wpool.tile([128, C], f32)
            nc.sync.dma_start(bias_s[:], bias_d[:])
            tvec_s = wpool.tile([128, 4], f32)
            nc.sync.dma_start(tvec_s[:], tvec_d[:])
            identb = wpool.tile([128, 128], bf16)
            nc.sync.dma_start(identb[:], identb_d[:])
            identf = wpool.tile([128, 128], f32)
            nc.sync.dma_start(identf[:], identf_d[:])
            wqth_s = wpool.tile([128, 6, C], bf16, name="wqth_s")
            nc.scalar.dma_start(
                wqth_s[:], wqth_d.ap().rearrange("(o p) f -> p o f", p=128)
            )
            wqtl_s = wpool.tile([128, 6, C], bf16, name="wqtl_s")
            nc.scalar.dma_start(
                wqtl_s[:], wqtl_d.ap().rearrange("(o p) f -> p o f", p=128)
            )

            # DRAM staging
            ssend = drpool.tile([4, HD, HD], f32)
            srecv = drpool.tile([4, HD, HD], f32)
            a2asend = [
                drpool.tile([8, 4, ECH, ROWS], bf16, name=f"a2as{g}")
                for g in range(NCHUNK)
            ]
            a2arecv = [
                drpool.tile([8, 4, ECH, ROWS], bf16, name=f"a2ar{g}")
                for g in range(NCHUNK)
            ]
            vband = [
                drpool.tile([2, 4 * ECH, N], bf16, name=f"vband{g}")
                for g in range(NCHUNK)
            ]

            # block-diag softmax matrix, zeroed early on idle gpsimd
            sm_s = smpool.tile([128, 6, C], bf16, name="sm_s")
            nc.gpsimd.memset(sm_s[:], 0)

            # g-pool: g_f (fp32 G), g_bh/g_bl (bf16 hi/lo split)
            g_f = gpool.tile([128, 6, C], f32, name="g_f")
            g_bh = gpool.tile([128, 6, C], bf16, name="g_bh")
            g_bl = gpool.tile([128, 6, C], bf16, name="g_bl")

            msb_cm = tc.tile_pool(name="msbpool", bufs=1)
            msbpool = msb_cm.__enter__()
            msb = msbpool.tile([128, 6, C], f32, name="msb")

            # ============ Phase G: M = 1/2 xh^T xh + xh^T xl2 ==============
            with tc.tile_pool(name="ps_g", bufs=1, space="PSUM") as ps_g:
                for half in range(2):
                    csl = slice(384 * half, 384 * (half + 1))
                    mps = [
                        ps_g.tile([128, 384], f32, tag=f"m{j}", name=f"mps{j}")
                        for j in range(6)
                    ]
                    for ch in range(NCH):
                        for j in range(6):
                            lhs = xh_s[:, ch, 128 * j : 128 * (j + 1)]
                            nc.tensor.matmul(
                                mps[j][:], lhs, xh_s[:, ch, csl],
                                start=(ch == 0), stop=False,
                                skip_group_check=True,
                            )
                            nc.tensor.matmul(
                                mps[j][:], lhs, xl2_s[:, ch, csl],
                                start=False, stop=(ch == NCH - 1),
                                skip_group_check=True,
                            )
                    for j in range(6):
                        if j % 2 == 0:
                            nc.vector.tensor_scalar_mul(
                                msb[:, j, csl], mps[j][:], 0.5
                            )
                        else:
                            nc.scalar.mul(msb[:, j, csl], mps[j][:], 0.5)

            # ---- G = M + M^T, then bf16 hi/lo split of G ----
            with tc.tile_pool(name="ps_sym", bufs=2, space="PSUM") as ps_sym:
                for i in range(6):
                    trow = ps_sym.tile([128, C], f32, tag="trow")
                    for j in range(6):
                        nc.tensor.matmul(
                            trow[:, 128 * j : 128 * (j + 1)],
                            msb[:, j, 128 * i : 128 * (i + 1)],
                            identf[:], is_transpose=True,
                            start=True, stop=True, skip_group_check=True,
                        )
                    nc.vector.tensor_add(g_f[:, i, :], msb[:, i, :], trow[:])
                    nc.scalar.copy(g_bh[:, i, :], g_f[:, i, :])
                    nc.vector.tensor_sub(g_bl[:, i, :], g_f[:, i, :], g_bh[:, i, :])

            msb_cm.__exit__(None, None, None)  # free msb (18KB)
            xio_cm.__exit__(None, None, None)  # free xh/xl2 (49KB)

            # ============ Phase A: A = G Wq^T, 3 bf16 hi/lo passes =========
            wka_cm = tc.tile_pool(name="wkapool", bufs=1)
            wkapool = wka_cm.__enter__()
            wkt_s = wkapool.tile([128, 6, C], f32, name="wkt_s")
            nc.scalar.dma_start(
                wkt_s[:], wkt_d.ap().rearrange("(o p) f -> p o f", p=128)
            )
            a_s = wkapool.tile([128, 6, C], f32, name="a_s")
            APASS = [("hh", None, None), ("hl", None, None), ("lh", None, None)]
            with tc.tile_pool(name="ps_a", bufs=1, space="PSUM") as ps_a:
                for half in range(2):
                    qsl = slice(384 * half, 384 * (half + 1))
                    for i in range(6):
                        ap_t = ps_a.tile(
                            [128, 384], f32, tag=f"a{i}", name=f"aps{i}"
                        )
                        nmm = 0
                        for gt, wt in ((g_bh, wqth_s), (g_bh, wqtl_s), (g_bl, wqth_s)):
                            for j in range(6):
                                nmm += 1
                                nc.tensor.matmul(
                                    ap_t[:],
                                    gt[:, j, 128 * i : 128 * (i + 1)],
                                    wt[:, j, qsl],
                                    start=(nmm == 1),
                                    stop=(nmm == 18),
                                    skip_group_check=True,
                                )
                        if i % 2 == 0:
                            nc.vector.tensor_copy(a_s[:, i, qsl], ap_t[:])
                        else:
                            nc.scalar.copy(a_s[:, i, qsl], ap_t[:])

            # ---- scores S^T_h = wk-contract A (fp32), then AllReduce ----
            sp_lo = scpool.tile([128, 4, HD], f32)
            sp_hi = scpool.tile([64, 4, HD], f32)
            with tc.tile_pool(name="ps_sc", bufs=2, space="PSUM") as ps_sc:
                for h in range(4):
                    hsl = slice(HD * h, HD * (h + 1))
                    st_lo = ps_sc.tile([128, HD], f32, tag="stlo", name="st_lo")
                    st_hi = ps_sc.tile([64, HD], f32, tag="sthi", name="st_hi")
                    for i in range(6):
                        nc.tensor.matmul(
                            st_lo[:],
                            wkt_s[:, i, HD * h : HD * h + 128],
                            a_s[:, i, hsl],
                            start=(i == 0), stop=(i == 5),
                            skip_group_check=True,
                        )
                    for i in range(6):
                        nc.tensor.matmul(
                            st_hi[:],
                            wkt_s[:, i, HD * h + 128 : HD * (h + 1)],
                            a_s[:, i, hsl],
                            start=(i == 0), stop=(i == 5),
                            skip_group_check=True,
                        )
                    nc.vector.tensor_copy(sp_lo[:, h, :], st_lo[:])
                    nc.scalar.copy(sp_hi[:, h, :], st_hi[:])
            for h in range(4):
                nc.sync.dma_start(ssend[h, 0:128, :], sp_lo[:, h, :])
                nc.sync.dma_start(ssend[h, 128:HD, :], sp_hi[:, h, :])
            nc.gpsimd.collective_compute(
                "AllReduce",
                mybir.AluOpType.add,
                replica_groups=RG,
                ins=[ssend.opt()],
                outs=[srecv.opt()],
            )
            wka_cm.__exit__(None, None, None)  # free wkt_s + a_s (36KB)
            g_cm.__exit__(None, None, None)  # free g tiles (36KB)

            # ============ Phase V (packed, covers score-AllReduce) ==========
            vt_cm = tc.tile_pool(name="vtpool", bufs=1)
            vtpool = vt_cm.__enter__()
            vt_s = vtpool.tile([128, 6, ROWS], bf16, name="vt_s")
            with tc.tile_pool(name="ps_v", bufs=1, space="PSUM") as ps_v:
                for w in range(NW):
                    nsl = slice(512 * w, 512 * (w + 1))
                    for k in range(6):
                        vp = ps_v.tile(
                            [128, 512], f32, tag=f"v{k}", name=f"vps{k}"
                        )
                        for cb in range(6):
                            nc.tensor.matmul(
                                vp[:],
                                wvt_s[:, cb, 128 * k : 128 * (k + 1)],
                                xht_s[:, cb, nsl],
                                start=(cb == 0), stop=(cb == 5),
                            )
                        if k % 2 == 0:
                            nc.vector.tensor_copy(vt_s[:, k, nsl], vp[:])
                        else:
                            nc.scalar.copy(vt_s[:, k, nsl], vp[:])

            # ---- softmax per head from reduced scores -> block-diag SM ----
            sr_lo, sr_hi = sp_lo, sp_hi
            for h in range(4):
                nc.sync.dma_start(sr_lo[:, h, :], srecv[h, 0:128, :])
                nc.sync.dma_start(sr_hi[:, h, :], srecv[h, 128:HD, :])
            for h in range(4):
                smt = {}
                for src_t, nrow in ((sr_lo, 128), (sr_hi, 64)):
                    ap_in = src_t[0:nrow, h, :]
                    mx = scpool.tile([nrow, 1], f32, tag=f"mx{nrow}", name="mx")
                    nc.vector.tensor_reduce(
                        mx[:], ap_in,
                        axis=mybir.AxisListType.X, op=mybir.AluOpType.max,
                    )
                    nmt = scpool.tile([nrow, 1], f32, tag=f"nm{nrow}", name="nmt")
                    nc.vector.tensor_mul(nmt[:], mx[:], tvec_s[:nrow, h : h + 1])
                    nc.vector.tensor_scalar_mul(nmt[:], nmt[:], -1.0)
                    p_t = scpool.tile([nrow, HD], f32, tag=f"p{nrow}", name="p_t")
                    ssum = scpool.tile([nrow, 1], f32, tag=f"s{nrow}", name="ssum")
                    nc.scalar.activation(
                        p_t[:], ap_in,
                        mybir.ActivationFunctionType.Exp,
                        bias=nmt[:], scale=tvec_s[:nrow, h : h + 1],
                        accum_out=ssum[:],
                    )
                    rcp = scpool.tile([nrow, 1], f32, tag=f"r{nrow}", name="rcp")
                    nc.vector.reciprocal(rcp[:], ssum[:])
                    smt_t = scpool.tile(
                        [nrow, HD], f32, tag=f"smt{nrow}", name="smt_t"
                    )
                    nc.vector.tensor_scalar_mul(smt_t[:], p_t[:], rcp[:])
                    smt[nrow] = smt_t
                with tc.tile_pool(name=f"ps_smt{h}", bufs=1, space="PSUM") as pst:
                    tlo = pst.tile([128, HD], f32, name="tlo")
                    thi = pst.tile([64, HD], f32, name="thi")
                    nc.tensor.matmul(
                        tlo[:, 0:128], smt[128][:, 0:128], identf[:],
                        is_transpose=True, start=True, stop=True,
                        skip_group_check=True,
                    )
                    nc.tensor.matmul(
                        tlo[:, 128:HD], smt[64][:, 0:128], identf[:64, 0:64],
                        is_transpose=True, start=True, stop=True,
                        skip_group_check=True,
                    )
                    nc.tensor.matmul(
                        thi[:, 0:128], smt[128][:, 128:HD], identf[:],
                        is_transpose=True, start=True, stop=True,
                        skip_group_check=True,
                    )
                    nc.tensor.matmul(
                        thi[:, 128:HD], smt[64][:, 128:HD], identf[:64, 0:64],
                        is_transpose=True, start=True, stop=True,
                        skip_group_check=True,
                    )
                    # scatter into packed block-diag rows 192h + a
                    csl = slice(HD * h, HD * (h + 1))
                    for (src, a0, nr) in ((tlo, 0, 128), (thi, 128, 64)):
                        r0 = HD * h + a0
                        placed = 0
                        while placed < nr:
                            j = (r0 + placed) // 128
                            p0 = (r0 + placed) % 128
                            cnt = min(128 - p0, nr - placed)
                            nc.vector.tensor_copy(
                                sm_s[p0 : p0 + cnt, j, csl],
                                src[placed : placed + cnt, :],
                            )
                            placed += cnt

            # ============ Phase X: x_caT = SM^T-contract vT (packed) ========
            XJ = {0: (0, 1), 1: (0, 1, 2), 2: (1, 2), 3: (3, 4), 4: (3, 4, 5), 5: (4, 5)}
            xc_cm = tc.tile_pool(name="xcpool", bufs=1)
            xcpool = xc_cm.__enter__()
            xcat_s = xcpool.tile([128, 6, ROWS], bf16, name="xcat_s")
            # strip pieces grouped by the xcat tile j they read, so each
            # tile's a2a send staging fires as soon as that tile is done
            strip_by_tile = {k: [] for k in range(6)}
            for g in range(NCHUNK):
                for i in range(8):
                    for h in range(4):
                        r0 = 192 * h + 24 * i + ECH * g
                        placed = 0
                        while placed < ECH:
                            j = (r0 + placed) // 128
                            p0 = (r0 + placed) % 128
                            cnt = min(128 - p0, ECH - placed)
                            strip_by_tile[j].append((g, i, h, placed, p0, cnt))
                            placed += cnt
            with tc.tile_pool(name="ps_x", bufs=1, space="PSUM") as ps_x:
                for k in range(6):
                    for w in range(NW):
                        nsl = slice(512 * w, 512 * (w + 1))
                        xp = ps_x.tile(
                            [128, 512], f32, tag=f"x{w}", name=f"xps{w}"
                        )
                        js = XJ[k]
                        for idx, j in enumerate(js):
                            nc.tensor.matmul(
                                xp[:],
                                sm_s[:, j, 128 * k : 128 * (k + 1)],
                                vt_s[:, j, nsl],
                                start=(idx == 0),
                                stop=(idx == len(js) - 1),
                            )
                        if w % 2 == 0:
                            nc.vector.tensor_copy(xcat_s[:, k, nsl], xp[:])
                        else:
                            nc.scalar.copy(xcat_s[:, k, nsl], xp[:])
                    for (g, i, h, placed, p0, cnt) in strip_by_tile[k]:
                        nc.sync.dma_start(
                            a2asend[g][i, h, placed : placed + cnt, :],
                            xcat_s[p0 : p0 + cnt, k, :],
                        )

            # ============ Tail: chunked a2a + projection pipeline ===========
            # chunk g: e-cols [24i+6g, 24i+6g+6) per dst i, all heads
            with (
                tc.tile_pool(name="zpool", bufs=3) as zpool,
                tc.tile_pool(name="opool", bufs=2) as opool,
                tc.tile_pool(name="ps_zt", bufs=2, space="PSUM") as ps_zt,
                tc.tile_pool(name="ps_o", bufs=2, space="PSUM") as ps_o,
            ):
                for g in range(NCHUNK):
                    nc.gpsimd.collective_compute(
                        "AllToAll",
                        mybir.AluOpType.bypass,
                        replica_groups=[list(range(8))],
                        ins=[a2asend[g].opt()],
                        outs=[a2arecv[g].opt()],
                    )
                    # assemble vband chunk: rows 4e+h, n from 4 senders
                    for b in range(2):
                        vb_v = vband[g][b].rearrange("(e h) n -> h e n", h=4)
                        for j in range(4):
                            eng = nc.sync if b == 0 else nc.scalar
                            eng.dma_start(
                                vb_v[:, :, ROWS * j : ROWS * (j + 1)],
                                a2arecv[g][4 * b + j],
                            )
                    # projection tiles: 2 per batch per chunk
                    for b in range(2):
                        for tt in range(ECH // 3):
                            t = (ECH // 3) * g + tt
                            z_nat = zpool.tile([128, C], bf16, tag="zn")
                            zeng = nc.scalar if b == 0 else nc.sync
                            zeng.dma_start(
                                z_nat[:],
                                vband[g][b, 12 * tt : 12 * (tt + 1), :],
                            )
                            ztp = ps_zt.tile([128, C], bf16, tag="ztp")
                            for j in range(6):
                                nc.tensor.matmul(
                                    ztp[:, 128 * j : 128 * (j + 1)],
                                    z_nat[:, 128 * j : 128 * (j + 1)],
                                    identb[:],
                                    is_transpose=True,
                                    start=True, stop=True,
                                    skip_group_check=True,
                                )
                            zt = zpool.tile([128, 6, 128], bf16, tag="zt")
                            if (b + tt) % 2 == 0:
                                nc.vector.tensor_copy(
                                    zt[:],
                                    ztp[:].rearrange("p (o f) -> p o f", f=128),
                                )
                            else:
                                nc.scalar.copy(
                                    zt[:],
                                    ztp[:].rearrange("p (o f) -> p o f", f=128),
                                )
                            o1 = ps_o.tile([128, 384], f32, tag="o1")
                            o2 = ps_o.tile([128, 384], f32, tag="o2")
                            for j in range(6):
                                nc.tensor.matmul(
                                    o1[:], zt[:, j, :], wpt_s[:, j, 0:384],
                                    start=(j == 0), stop=(j == 5),
                                )
                            for j in range(6):
                                nc.tensor.matmul(
                                    o2[:], zt[:, j, :], wpt_s[:, j, 384:C],
                                    start=(j == 0), stop=(j == 5),
                                )
                            out_sb = opool.tile([128, C], bf16, tag="ob")
                            nc.vector.tensor_add(
                                out_sb[:, 0:384], o1[:], bias_s[:, 0:384]
                            )
                            nc.vector.tensor_add(
                                out_sb[:, 384:C], o2[:], bias_s[:, 384:C]
                            )
                            oeng = nc.sync if b == 0 else nc.scalar
                            oeng.dma_start(
                                out_d[b, 128 * t : 128 * (t + 1), :], out_sb[:]
                            )

            xc_cm.__exit__(None, None, None)
            vt_cm.__exit__(None, None, None)
            sc_cm.__exit__(None, None, None)
            xht_cm.__exit__(None, None, None)
            sm_cm.__exit__(None, None, None)

    nc.compile()
    return nc


def _get_nc():
    if "nc" not in _cached:
        _cached["nc"] = _build()
    return _cached["nc"]


def _prep_in_maps(x, w_qkv, temperature, w_proj, b_proj):
    import ml_dtypes

    bf = ml_dtypes.bfloat16
    x = np.ascontiguousarray(np.asarray(x, dtype=np.float32))
    w_qkv = np.asarray(w_qkv, dtype=np.float32)
    temperature = np.asarray(temperature, dtype=np.float32)
    w_proj = np.asarray(w_proj, dtype=np.float32)
    b_proj = np.asarray(b_proj, dtype=np.float32)

    wqt = np.ascontiguousarray(w_qkv[0:C].T)
    wqth = wqt.astype(bf)
    wqtl = (wqt - wqth.astype(np.float32)).astype(bf)
    wkt = np.ascontiguousarray(w_qkv[C : 2 * C].T)
    wvt = np.ascontiguousarray(w_qkv[2 * C : 3 * C].T).astype(bf)
    wpt = np.ascontiguousarray(w_proj.T).astype(bf)

    bias = np.ascontiguousarray(np.broadcast_to(b_proj, (128, C)))
    tvec = np.broadcast_to(
        temperature.reshape(1, H).astype(np.float32), (128, H)
    ).copy()
    identb = np.eye(128, dtype=np.float32).astype(bf)
    identf = np.eye(128, dtype=np.float32)

    in_maps = []
    for c in range(NCORE):
        b, r = c // 4, c % 4
        xs = x[b, ROWS * r : ROWS * (r + 1), :]
        xh = xs.astype(bf)
        xl2 = ((xs - xh.astype(np.float32)) * 2.0).astype(bf)
        in_maps.append(
            {
                "xh": np.ascontiguousarray(xh),
                "xl2": np.ascontiguousarray(xl2),
                "xht": np.ascontiguousarray(xs.T).astype(bf),
                "wqth": wqth,
                "wqtl": wqtl,
                "wkt": wkt,
                "wvt": wvt,
                "wpt": wpt,
                "bias": bias,
                "tvec": tvec,
                "identb": identb,
                "identf": identf,
            }
        )
    return in_maps


def kernel(x, w_qkv, temperature, w_proj, b_proj):
    from concourse.bass_utils import run_bass_kernel_spmd

    nc = _get_nc()
    in_maps = _prep_in_maps(x, w_qkv, temperature, w_proj, b_proj)
    res = run_bass_kernel_spmd(nc, in_maps, core_ids=list(range(NCORE)))
    out = np.empty((B, N, C), np.float32)
    for c in range(NCORE):
        o = res.results[c]["out"]  # [2, 1024, C] bf16
        for b in range(B):
            out[b, 1024 * c : 1024 * (c + 1), :] = o[b].astype(np.float32)
    return out


# revision 34
# speedup vs baseline: 1.2552x; 1.0279x over previous
"""ChannelAttn Trainium2 kernel v3.1: bf16 hi/lo score path + packed value
path + chunked AllToAll tail.

Sharding: core c handles batch b=c//4, rows [2048*(c%4), 2048*(c%4+1)).
Replica groups per batch for score AllReduce: [[0,1,2,3],[4,5,6,7]].

Score path (softmax logits need ~fp32 accuracy; every single-precision
shortcut measurably breaks the 2e-2 gate):
  M = 1/2 xh^T xh + xh^T xl2 via bf16 hi/lo (xh=bf16(x), xl2=bf16(2(x-xh)),
  host-prepped), scaled 0.5 on the PSUM->SBUF copy; G = M + M^T locally
  (fp32 PE transposes). A = G Wq^T in THREE bf16 passes using hi/lo splits
  of both G (on-chip DVE split) and Wq^T (host): gh*wh + gh*wl + gl*wh
  (error ~5e-4 in logits). Scores S^T_h = wk-contract A in true fp32.
  AllReduce partial scores over the batch group (hidden behind phase V);
  softmax over the free dim; PE-transpose pieces into a block-diagonal
  SM [768x768, packed (h,d) x (h,e)] bf16 (zero-filled by gpsimd memset).

Value path: vT[(h,d) packed, n] = Wv^T-contract xhT where xhT is shipped
pre-transposed bf16 from the host (no on-chip transpose phase);
x_caT = SM^T-contract vT, 14 matmuls per 512-wide window (block-diag
skips zero blocks); AllToAll in 4 e-chunks so each chunk's projection
tiles (z transpose + Wp matmul + bias) pipeline behind later chunks.

Shapes hardcoded: B=2, N=8192, C=768, H=4, HD=192.
"""

import sys

sys.path.insert(0, "/opt/trn_rl_repo")

import numpy as np

B, N, C, H = 2, 8192, 768, 4
HD = C // H  # 192
NCORE = 8
ROWS = N // 4  # 2048 rows per core (of one batch)
NCH = ROWS // 128  # 16 chunks
NW = ROWS // 512  # 4 windows
NCHUNK = 2  # a2a chunks (e-columns of each dst's 24 split across chunks)
ECH = 24 // NCHUNK  # e-cols per dst per chunk

_cached = {}


def _build():
    import concourse.bacc as bacc
    import concourse.mybir as mybir
    import concourse.tile as tile

    f32 = mybir.dt.float32
    bf16 = mybir.dt.bfloat16

    nc = bacc.Bacc("TRN2", target_bir_lowering=False, debug=False)

    xh_d = nc.dram_tensor("xh", [128, NCH * C], bf16, kind="ExternalInput")
    xl2_d = nc.dram_tensor("xl2", [128, NCH * C], bf16, kind="ExternalInput")
    xht_d = nc.dram_tensor("xht", [128, 6 * ROWS], bf16, kind="ExternalInput")
    wqth_d = nc.dram_tensor("wqth", [128, 6 * C], bf16, kind="ExternalInput")
    wqtl_d = nc.dram_tensor("wqtl", [128, 6 * C], bf16, kind="ExternalInput")
    wkt_d = nc.dram_tensor("wkt", [128, 6 * C], f32, kind="ExternalInput")
    wvt_d = nc.dram_tensor("wvt", [128, 6 * C], bf16, kind="ExternalInput")
    wpt_d = nc.dram_tensor("wpt", [128, 6 * C], bf16, kind="ExternalInput")
    bias_d = nc.dram_tensor("bias", [128, C], f32, kind="ExternalInput")
    tvec_d = nc.dram_tensor("tvec", [128, 4], f32, kind="ExternalInput")
    identb_d = nc.dram_tensor("identb", [128, 128], bf16, kind="ExternalInput")
    identf_d = nc.dram_tensor("identf", [128, 128], f32, kind="ExternalInput")
    out_d = nc.dram_tensor("out", [2, 1024, C], bf16, kind="ExternalOutput")

    RG = [[0, 1, 2, 3], [4, 5, 6, 7]]

    with tile.TileContext(nc) as tc:
        with (
            tc.tile_pool(name="wpool", bufs=1) as wpool,
            tc.tile_pool(name="drpool", bufs=1, space="DRAM") as drpool,
        ):
            # ---- long-lived pools (stack order = reverse close order) ----
            sm_cm = tc.tile_pool(name="smpool", bufs=1)
            smpool = sm_cm.__enter__()
            xht_cm = tc.tile_pool(name="xhtpool", bufs=1)
            xhtpool = xht_cm.__enter__()
            sc_cm = tc.tile_pool(name="scpool", bufs=1)
            scpool = sc_cm.__enter__()
            g_cm = tc.tile_pool(name="gpool", bufs=1)
            gpool = g_cm.__enter__()
            xio_cm = tc.tile_pool(name="xio", bufs=1)
            xio = xio_cm.__enter__()

            # ---- x hi/lo DMAs issued FIRST so phase G starts early ----
            xh_s = xio.tile([128, NCH, C], bf16, name="xh_s")
            xl2_s = xio.tile([128, NCH, C], bf16, name="xl2_s")
            for g in range(4):
                eng_a = nc.sync if g % 2 == 0 else nc.scalar
                eng_b = nc.scalar if g % 2 == 0 else nc.sync
                eng_a.dma_start(
                    xh_s[:, 4 * g : 4 * (g + 1), :],
                    xh_d[:, 4 * g * C : 4 * (g + 1) * C],
                )
                eng_b.dma_start(
                    xl2_s[:, 4 * g : 4 * (g + 1), :],
                    xl2_d[:, 4 * g * C : 4 * (g + 1) * C],
                )
            xht_s = xhtpool.tile([128, 6, ROWS], bf16, name="xht_s")
            nc.scalar.dma_start(xht_s[:], xht_d.ap())

            # ---- weights / constants (whole-kernel) ----
            wvt_s = wpool.tile([128, 6, C], bf16)
            nc.sync.dma_start(wvt_s[:], wvt_d.ap())
            wpt_s = wpool.tile([128, 6, C], bf16)
            nc.sync.dma_start(wpt_s[:], wpt_d.ap())
            bias_s = wpool.tile([128, C], f32)
            nc.sync.dma_start(bias_s[:], bias_d[:])
            tvec_s = wpool.tile([128, 4], f32)
            nc.sync.dma_start(tvec_s[:], tvec_d[:])
            identb = wpool.tile([128, 128], bf16)
            nc.sync.dma_start(identb[:], identb_d[:])
            identf = wpool.tile([128, 128], f32)
            nc.sync.dma_start(identf[:], identf_d[:])
            wqth_s = wpool.tile([128, 6, C], bf16, name="wqth_s")
            nc.scalar.dma_start(wqth_s[:], wqth_d.ap())
            wqtl_s = wpool.tile([128, 6, C], bf16, name="wqtl_s")
            nc.scalar.dma_start(wqtl_s[:], wqtl_d.ap())

            # DRAM staging
            ssend = drpool.tile([4, HD, HD], f32)
            srecv = drpool.tile([4, HD, HD], f32)
            a2asend = [
                drpool.tile([8, 4, ECH, ROWS], bf16, name=f"a2as{g}")
                for g in range(NCHUNK)
            ]
            a2arecv = [
                drpool.tile([8, 4, ECH, ROWS], bf16, name=f"a2ar{g}")
                for g in range(NCHUNK)
            ]
            vband = [
                drpool.tile([2, 4 * ECH, N], bf16, name=f"vband{g}")
                for g in range(NCHUNK)
            ]

            # block-diag softmax matrix, zeroed early on idle gpsimd
            sm_s = smpool.tile([128, 6, C], bf16, name="sm_s")
            nc.gpsimd.memset(sm_s[:], 0)

            # g-pool: g_f (fp32 G), g_bh/g_bl (bf16 hi/lo split)
            g_f = gpool.tile([128, 6, C], f32, name="g_f")
            g_bh = gpool.tile([128, 6, C], bf16, name="g_bh")
            g_bl = gpool.tile([128, 6, C], bf16, name="g_bl")

            msb_cm = tc.tile_pool(name="msbpool", bufs=1)
            msbpool = msb_cm.__enter__()
            msb = msbpool.tile([128, 6, C], f32, name="msb")

            # ============ Phase G: M = 1/2 xh^T xh + xh^T xl2 ==============
            with tc.tile_pool(name="ps_g", bufs=1, space="PSUM") as ps_g:
                for half in range(2):
                    csl = slice(384 * half, 384 * (half + 1))
                    mps = [
                        ps_g.tile([128, 384], f32, tag=f"m{j}", name=f"mps{j}")
                        for j in range(6)
                    ]
                    for ch in range(NCH):
                        for j in range(6):
                            lhs = xh_s[:, ch, 128 * j : 128 * (j + 1)]
                            nc.tensor.matmul(
                                mps[j][:], lhs, xh_s[:, ch, csl],
                                start=(ch == 0), stop=False,
                                skip_group_check=True,
                            )
                            nc.tensor.matmul(
                                mps[j][:], lhs, xl2_s[:, ch, csl],
                                start=False, stop=(ch == NCH - 1),
                                skip_group_check=True,
                            )
                    for j in range(6):
                        if j % 2 == 0:
                            nc.vector.tensor_scalar_mul(
                                msb[:, j, csl], mps[j][:], 0.5
                            )
                        else:
                            nc.scalar.mul(msb[:, j, csl], mps[j][:], 0.5)

            # ---- G = M + M^T, then bf16 hi/lo split of G ----
            with tc.tile_pool(name="ps_sym", bufs=2, space="PSUM") as ps_sym:
                for i in range(6):
                    trow = ps_sym.tile([128, C], f32, tag="trow")
                    for j in range(6):
                        nc.tensor.matmul(
                            trow[:, 128 * j : 128 * (j + 1)],
                            msb[:, j, 128 * i : 128 * (i + 1)],
                            identf[:], is_transpose=True,
                            start=True, stop=True, skip_group_check=True,
                        )
                    nc.vector.tensor_add(g_f[:, i, :], msb[:, i, :], trow[:])
                    nc.scalar.copy(g_bh[:, i, :], g_f[:, i, :])
                    nc.vector.tensor_sub(g_bl[:, i, :], g_f[:, i, :], g_bh[:, i, :])

            msb_cm.__exit__(None, None, None)  # free msb (18KB)
            xio_cm.__exit__(None, None, None)  # free xh/xl2 (49KB)

            # ============ Phase A: A = G Wq^T, 3 bf16 hi/lo passes =========
            wka_cm = tc.tile_pool(name="wkapool", bufs=1)
            wkapool = wka_cm.__enter__()
            wkt_s = wkapool.tile([128, 6, C], f32, name="wkt_s")
            nc.scalar.dma_start(wkt_s[:], wkt_d.ap())
            a_s = wkapool.tile([128, 6, C], f32, name="a_s")
            APASS = [("hh", None, None), ("hl", None, None), ("lh", None, None)]
            with tc.tile_pool(name="ps_a", bufs=1, space="PSUM") as ps_a:
                for half in range(2):
                    qsl = slice(384 * half, 384 * (half + 1))
                    for i in range(6):
                        ap_t = ps_a.tile(
                            [128, 384], f32, tag=f"a{i}", name=f"aps{i}"
                        )
                        nmm = 0
                        for gt, wt in ((g_bh, wqth_s), (g_bh, wqtl_s), (g_bl, wqth_s)):
                            for j in range(6):
                                nmm += 1
                                nc.tensor.matmul(
                                    ap_t[:],
                                    gt[:, j, 128 * i : 128 * (i + 1)],
                                    wt[:, j, qsl],
                                    start=(nmm == 1),
                                    stop=(nmm == 18),
                                    skip_group_check=True,
                                )
                        if i % 2 == 0:
                            nc.vector.tensor_copy(a_s[:, i, qsl], ap_t[:])
                        else:
                            nc.scalar.copy(a_s[:, i, qsl], ap_t[:])

            # ---- scores S^T_h = wk-contract A (fp32), then AllReduce ----
            sp_lo = scpool.tile([128, 4, HD], f32)
            sp_hi = scpool.tile([64, 4, HD], f32)
            with tc.tile_pool(name="ps_sc", bufs=2, space="PSUM") as ps_sc:
                for h in range(4):
                    hsl = slice(HD * h, HD * (h + 1))
                    st_lo = ps_sc.tile([128, HD], f32, tag="stlo", name="st_lo")
                    st_hi = ps_sc.tile([64, HD], f32, tag="sthi", name="st_hi")
                    for i in range(6):
                        nc.tensor.matmul(
                            st_lo[:],
                            wkt_s[:, i, HD * h : HD * h + 128],
                            a_s[:, i, hsl],
                            start=(i == 0), stop=(i == 5),
                            skip_group_check=True,
                        )
                    for i in range(6):
                        nc.tensor.matmul(
                            st_hi[:],
                            wkt_s[:, i, HD * h + 128 : HD * (h + 1)],
                            a_s[:, i, hsl],
                            start=(i == 0), stop=(i == 5),
                            skip_group_check=True,
                        )
                    nc.vector.tensor_copy(sp_lo[:, h, :], st_lo[:])
                    nc.scalar.copy(sp_hi[:, h, :], st_hi[:])
            for h in range(4):
                nc.sync.dma_start(ssend[h, 0:128, :], sp_lo[:, h, :])
                nc.sync.dma_start(ssend[h, 128:HD, :], sp_hi[:, h, :])
            nc.gpsimd.collective_compute(
                "AllReduce",
                mybir.AluOpType.add,
                replica_groups=RG,
                ins=[ssend.opt()],
                outs=[srecv.opt()],
            )
            wka_cm.__exit__(None, None, None)  # free wkt_s + a_s (36KB)
            g_cm.__exit__(None, None, None)  # free g tiles (36KB)

            # ============ Phase V (packed, covers score-AllReduce) ==========
            vt_cm = tc.tile_pool(name="vtpool", bufs=1)
            vtpool = vt_cm.__enter__()
            vt_s = vtpool.tile([128, 6, ROWS], bf16, name="vt_s")
            with tc.tile_pool(name="ps_v", bufs=1, space="PSUM") as ps_v:
                for w in range(NW):
                    nsl = slice(512 * w, 512 * (w + 1))
                    for k in range(6):
                        vp = ps_v.tile(
                            [128, 512], f32, tag=f"v{k}", name=f"vps{k}"
                        )
                        for cb in range(6):
                            nc.tensor.matmul(
                                vp[:],
                                wvt_s[:, cb, 128 * k : 128 * (k + 1)],
                                xht_s[:, cb, nsl],
                                start=(cb == 0), stop=(cb == 5),
                            )
                        if k % 2 == 0:
                            nc.vector.tensor_copy(vt_s[:, k, nsl], vp[:])
                        else:
                            nc.scalar.copy(vt_s[:, k, nsl], vp[:])

            # ---- softmax per head from reduced scores -> block-diag SM ----
            sr_lo, sr_hi = sp_lo, sp_hi
            for h in range(4):
                nc.sync.dma_start(sr_lo[:, h, :], srecv[h, 0:128, :])
                nc.sync.dma_start(sr_hi[:, h, :], srecv[h, 128:HD, :])
            for h in range(4):
                smt = {}
                for src_t, nrow in ((sr_lo, 128), (sr_hi, 64)):
                    ap_in = src_t[0:nrow, h, :]
                    mx = scpool.tile([nrow, 1], f32, tag=f"mx{nrow}", name="mx")
                    nc.vector.tensor_reduce(
                        mx[:], ap_in,
                        axis=mybir.AxisListType.X, op=mybir.AluOpType.max,
                    )
                    nmt = scpool.tile([nrow, 1], f32, tag=f"nm{nrow}", name="nmt")
                    nc.vector.tensor_mul(nmt[:], mx[:], tvec_s[:nrow, h : h + 1])
                    nc.vector.tensor_scalar_mul(nmt[:], nmt[:], -1.0)
                    p_t = scpool.tile([nrow, HD], f32, tag=f"p{nrow}", name="p_t")
                    ssum = scpool.tile([nrow, 1], f32, tag=f"s{nrow}", name="ssum")
                    nc.scalar.activation(
                        p_t[:], ap_in,
                        mybir.ActivationFunctionType.Exp,
                        bias=nmt[:], scale=tvec_s[:nrow, h : h + 1],
                        accum_out=ssum[:],
                    )
                    rcp = scpool.tile([nrow, 1], f32, tag=f"r{nrow}", name="rcp")
                    nc.vector.reciprocal(rcp[:], ssum[:])
                    smt_t = scpool.tile(
                        [nrow, HD], f32, tag=f"smt{nrow}", name="smt_t"
                    )
                    nc.vector.tensor_scalar_mul(smt_t[:], p_t[:], rcp[:])
                    smt[nrow] = smt_t
                with tc.tile_pool(name=f"ps_smt{h}", bufs=1, space="PSUM") as pst:
                    tlo = pst.tile([128, HD], f32, name="tlo")
                    thi = pst.tile([64, HD], f32, name="thi")
                    nc.tensor.matmul(
                        tlo[:, 0:128], smt[128][:, 0:128], identf[:],
                        is_transpose=True, start=True, stop=True,
                        skip_group_check=True,
                    )
                    nc.tensor.matmul(
                        tlo[:, 128:HD], smt[64][:, 0:128], identf[:64, 0:64],
                        is_transpose=True, start=True, stop=True,
                        skip_group_check=True,
                    )
                    nc.tensor.matmul(
                        thi[:, 0:128], smt[128][:, 128:HD], identf[:],
                        is_transpose=True, start=True, stop=True,
                        skip_group_check=True,
                    )
                    nc.tensor.matmul(
                        thi[:, 128:HD], smt[64][:, 128:HD], identf[:64, 0:64],
                        is_transpose=True, start=True, stop=True,
                        skip_group_check=True,
                    )
                    # scatter into packed block-diag rows 192h + a
                    csl = slice(HD * h, HD * (h + 1))
                    for (src, a0, nr) in ((tlo, 0, 128), (thi, 128, 64)):
                        r0 = HD * h + a0
                        placed = 0
                        while placed < nr:
                            j = (r0 + placed) // 128
                            p0 = (r0 + placed) % 128
                            cnt = min(128 - p0, nr - placed)
                            nc.vector.tensor_copy(
                                sm_s[p0 : p0 + cnt, j, csl],
                                src[placed : placed + cnt, :],
                            )
                            placed += cnt

            # ============ Phase X: x_caT = SM^T-contract vT (packed) ========
            XJ = {0: (0, 1), 1: (0, 1, 2), 2: (1, 2), 3: (3, 4), 4: (3, 4, 5), 5: (4, 5)}
            xc_cm = tc.tile_pool(name="xcpool", bufs=1)
            xcpool = xc_cm.__enter__()
            xcat_s = xcpool.tile([128, 6, ROWS], bf16, name="xcat_s")
            # strip pieces grouped by the xcat tile j they read, so each
            # tile's a2a send staging fires as soon as that tile is done
            strip_by_tile = {k: [] for k in range(6)}
            for g in range(NCHUNK):
                for i in range(8):
                    for h in range(4):
                        r0 = 192 * h + 24 * i + ECH * g
                        placed = 0
                        while placed < ECH:
                            j = (r0 + placed) // 128
                            p0 = (r0 + placed) % 128
                            cnt = min(128 - p0, ECH - placed)
                            strip_by_tile[j].append((g, i, h, placed, p0, cnt))
                            placed += cnt
            with tc.tile_pool(name="ps_x", bufs=1, space="PSUM") as ps_x:
                for k in range(6):
                    for w in range(NW):
                        nsl = slice(512 * w, 512 * (w + 1))
                        xp = ps_x.tile(
                            [128, 512], f32, tag=f"x{w}", name=f"xps{w}"
                        )
                        js = XJ[k]
                        for idx, j in enumerate(js):
                            nc.tensor.matmul(
                                xp[:],
                                sm_s[:, j, 128 * k : 128 * (k + 1)],
                                vt_s[:, j, nsl],
                                start=(idx == 0),
                                stop=(idx == len(js) - 1),
                            )
                        if w % 2 == 0:
                            nc.vector.tensor_copy(xcat_s[:, k, nsl], xp[:])
                        else:
                            nc.scalar.copy(xcat_s[:, k, nsl], xp[:])
                    for (g, i, h, placed, p0, cnt) in strip_by_tile[k]:
                        nc.sync.dma_start(
                            a2asend[g][i, h, placed : placed + cnt, :],
                            xcat_s[p0 : p0 + cnt, k, :],
                        )

            # ============ Tail: chunked a2a + projection pipeline ===========
            # chunk g: e-cols [24i+6g, 24i+6g+6) per dst i, all heads
            with (
                tc.tile_pool(name="zpool", bufs=3) as zpool,
                tc.tile_pool(name="opool", bufs=2) as opool,
                tc.tile_pool(name="ps_zt", bufs=2, space="PSUM") as ps_zt,
                tc.tile_pool(name="ps_o", bufs=2, space="PSUM") as ps_o,
            ):
                for g in range(NCHUNK):
                    nc.gpsimd.collective_compute(
                        "AllToAll",
                        mybir.AluOpType.bypass,
                        replica_groups=[list(range(8))],
                        ins=[a2asend[g].opt()],
                        outs=[a2arecv[g].opt()],
                    )
                    # assemble vband chunk: rows 4e+h, n from 4 senders
                    for b in range(2):
                        vb_v = vband[g][b].rearrange("(e h) n -> h e n", h=4)
                        for j in range(4):
                            eng = nc.sync if b == 0 else nc.scalar
                            eng.dma_start(
                                vb_v[:, :, ROWS * j : ROWS * (j + 1)],
                                a2arecv[g][4 * b + j],
                            )
                    # projection tiles: 2 per batch per chunk
                    for b in range(2):
                        for tt in range(ECH // 3):
                            t = (ECH // 3) * g + tt
                            z_nat = zpool.tile([128, C], bf16, tag="zn")
                            zeng = nc.scalar if b == 0 else nc.sync
                            zeng.dma_start(
                                z_nat[:],
                                vband[g][b, 12 * tt : 12 * (tt + 1), :],
                            )
                            ztp = ps_zt.tile([128, C], bf16, tag="ztp")
                            for j in range(6):
                                nc.tensor.matmul(
                                    ztp[:, 128 * j : 128 * (j + 1)],
                                    z_nat[:, 128 * j : 128 * (j + 1)],
                                    identb[:],
                                    is_transpose=True,
                                    start=True, stop=True,
                                    skip_group_check=True,
                                )
                            zt = zpool.tile([128, 6, 128], bf16, tag="zt")
                            if (b + tt) % 2 == 0:
                                nc.vector.tensor_copy(
                                    zt[:],
                                    ztp[:].rearrange("p (o f) -> p o f", f=128),
                                )
                            else:
                                nc.scalar.copy(
                                    zt[:],
                                    ztp[:].rearrange("p (o f) -> p o f", f=128),
                                )
                            o1 = ps_o.tile([128, 384], f32, tag="o1")
                            o2 = ps_o.tile([128, 384], f32, tag="o2")
                            for j in range(6):
                                nc.tensor.matmul(
                                    o1[:], zt[:, j, :], wpt_s[:, j, 0:384],
                                    start=(j == 0), stop=(j == 5),
                                )
                            for j in range(6):
                                nc.tensor.matmul(
                                    o2[:], zt[:, j, :], wpt_s[:, j, 384:C],
                                    start=(j == 0), stop=(j == 5),
                                )
                            out_sb = opool.tile([128, C], bf16, tag="ob")
                            nc.vector.tensor_add(
                                out_sb[:, 0:384], o1[:], bias_s[:, 0:384]
                            )
                            nc.vector.tensor_add(
                                out_sb[:, 384:C], o2[:], bias_s[:, 384:C]
                            )
                            oeng = nc.sync if b == 0 else nc.scalar
                            oeng.dma_start(
                                out_d[b, 128 * t : 128 * (t + 1), :], out_sb[:]
                            )

            xc_cm.__exit__(None, None, None)
            vt_cm.__exit__(None, None, None)
            sc_cm.__exit__(None, None, None)
            xht_cm.__exit__(None, None, None)
            sm_cm.__exit__(None, None, None)

    nc.compile()
    return nc


def _get_nc():
    if "nc" not in _cached:
        _cached["nc"] = _build()
    return _cached["nc"]


def _pretile(a, p=128):
    # [o*p, f] -> [p, o*f], matching SBUF tile [p, o, f]
    rows, f = a.shape
    o = rows // p
    return np.ascontiguousarray(
        a.reshape(o, p, f).transpose(1, 0, 2).reshape(p, o * f)
    )


def _prep_in_maps(x, w_qkv, temperature, w_proj, b_proj):
    import ml_dtypes

    bf = ml_dtypes.bfloat16
    x = np.ascontiguousarray(np.asarray(x, dtype=np.float32))
    w_qkv = np.asarray(w_qkv, dtype=np.float32)
    temperature = np.asarray(temperature, dtype=np.float32)
    w_proj = np.asarray(w_proj, dtype=np.float32)
    b_proj = np.asarray(b_proj, dtype=np.float32)

    wqt = np.ascontiguousarray(w_qkv[0:C].T)
    wqth = wqt.astype(bf)
    wqtl = (wqt - wqth.astype(np.float32)).astype(bf)
    wqth = _pretile(wqth)
    wqtl = _pretile(wqtl)
    wkt = _pretile(np.ascontiguousarray(w_qkv[C : 2 * C].T))
    wvt = _pretile(np.ascontiguousarray(w_qkv[2 * C : 3 * C].T).astype(bf))
    wpt = _pretile(np.ascontiguousarray(w_proj.T).astype(bf))

    bias = np.ascontiguousarray(np.broadcast_to(b_proj, (128, C)))
    tvec = np.broadcast_to(
        temperature.reshape(1, H).astype(np.float32), (128, H)
    ).copy()
    identb = np.eye(128, dtype=np.float32).astype(bf)
    identf = np.eye(128, dtype=np.float32)

    in_maps = []
    for c in range(NCORE):
        b, r = c // 4, c % 4
        xs = x[b, ROWS * r : ROWS * (r + 1), :]
        xh = xs.astype(bf)
        xl2 = ((xs - xh.astype(np.float32)) * 2.0).astype(bf)
        in_maps.append(
            {
                "xh": _pretile(np.ascontiguousarray(xh)),
                "xl2": _pretile(np.ascontiguousarray(xl2)),
                "xht": _pretile(np.ascontiguousarray(xs.T).astype(bf)),
                "wqth": wqth,
                "wqtl": wqtl,
                "wkt": wkt,
                "wvt": wvt,
                "wpt": wpt,
                "bias": bias,
                "tvec": tvec,
                "identb": identb,
                "identf": identf,
            }
        )
    return in_maps


def kernel(x, w_qkv, temperature, w_proj, b_proj):
    from concourse.bass_utils import run_bass_kernel_spmd

    nc = _get_nc()
    in_maps = _prep_in_maps(x, w_qkv, temperature, w_proj, b_proj)
    res = run_bass_kernel_spmd(nc, in_maps, core_ids=list(range(NCORE)))
    out = np.empty((B, N, C), np.float32)
    for c in range(NCORE):
        o = res.results[c]["out"]  # [2, 1024, C] bf16
        for b in range(B):
            out[b, 1024 * c : 1024 * (c + 1), :] = o[b].astype(np.float32)
    return out


# revision 35
# speedup vs baseline: 1.3186x; 1.0505x over previous
"""ChannelAttn Trainium2 kernel v3.1: bf16 hi/lo score path + packed value
path + chunked AllToAll tail.

Sharding: core c handles batch b=c//4, rows [2048*(c%4), 2048*(c%4+1)).
Replica groups per batch for score AllReduce: [[0,1,2,3],[4,5,6,7]].

Score path (softmax logits need ~fp32 accuracy; every single-precision
shortcut measurably breaks the 2e-2 gate):
  M = 1/2 xh^T xh + xh^T xl2 via bf16 hi/lo (xh=bf16(x), xl2=bf16(2(x-xh)),
  host-prepped), scaled 0.5 on the PSUM->SBUF copy; G = M + M^T locally
  (fp32 PE transposes). A = G Wq^T in THREE bf16 passes using hi/lo splits
  of both G (on-chip DVE split) and Wq^T (host): gh*wh + gh*wl + gl*wh
  (error ~5e-4 in logits). Scores S^T_h = wk-contract A in true fp32.
  AllReduce partial scores over the batch group (hidden behind phase V);
  softmax over the free dim; PE-transpose pieces into a block-diagonal
  SM [768x768, packed (h,d) x (h,e)] bf16 (zero-filled by gpsimd memset).

Value path: vT[(h,d) packed, n] = Wv^T-contract xhT where xhT is shipped
pre-transposed bf16 from the host (no on-chip transpose phase);
x_caT = SM^T-contract vT, 14 matmuls per 512-wide window (block-diag
skips zero blocks); AllToAll in 4 e-chunks so each chunk's projection
tiles (z transpose + Wp matmul + bias) pipeline behind later chunks.

Shapes hardcoded: B=2, N=8192, C=768, H=4, HD=192.
"""

import sys

sys.path.insert(0, "/opt/trn_rl_repo")

import numpy as np

B, N, C, H = 2, 8192, 768, 4
HD = C // H  # 192
NCORE = 8
ROWS = N // 4  # 2048 rows per core (of one batch)
NCH = ROWS // 128  # 16 chunks
NW = ROWS // 512  # 4 windows
NCHUNK = 2  # a2a chunks (e-columns of each dst's 24 split across chunks)
ECH = 24 // NCHUNK  # e-cols per dst per chunk

_cached = {}


def _build():
    import concourse.bacc as bacc
    import concourse.mybir as mybir
    import concourse.tile as tile

    f32 = mybir.dt.float32
    bf16 = mybir.dt.bfloat16

    nc = bacc.Bacc("TRN2", target_bir_lowering=False, debug=False)

    xh_d = nc.dram_tensor("xh", [128, NCH * C], bf16, kind="ExternalInput")
    xl2_d = nc.dram_tensor("xl2", [128, NCH * C], bf16, kind="ExternalInput")
    xht_d = nc.dram_tensor("xht", [128, 6 * ROWS], bf16, kind="ExternalInput")
    wqth_d = nc.dram_tensor("wqth", [128, 6 * C], bf16, kind="ExternalInput")
    wqtl_d = nc.dram_tensor("wqtl", [128, 6 * C], bf16, kind="ExternalInput")
    wkt_d = nc.dram_tensor("wkt", [128, 6 * C], f32, kind="ExternalInput")
    wvt_d = nc.dram_tensor("wvt", [128, 6 * C], bf16, kind="ExternalInput")
    wpt_d = nc.dram_tensor("wpt", [128, 6 * C], bf16, kind="ExternalInput")
    bias_d = nc.dram_tensor("bias", [128, C], f32, kind="ExternalInput")
    tvec_d = nc.dram_tensor("tvec", [128, 4], f32, kind="ExternalInput")
    identb_d = nc.dram_tensor("identb", [128, 128], bf16, kind="ExternalInput")
    identf_d = nc.dram_tensor("identf", [128, 128], f32, kind="ExternalInput")
    out_d = nc.dram_tensor("out", [2, 1024, C], bf16, kind="ExternalOutput")

    RG = [[0, 1, 2, 3], [4, 5, 6, 7]]

    with tile.TileContext(nc) as tc:
        with (
            tc.tile_pool(name="wpool", bufs=1) as wpool,
            tc.tile_pool(name="drpool", bufs=1, space="DRAM") as drpool,
        ):
            # ---- long-lived pools (stack order = reverse close order) ----
            sm_cm = tc.tile_pool(name="smpool", bufs=1)
            smpool = sm_cm.__enter__()
            xht_cm = tc.tile_pool(name="xhtpool", bufs=1)
            xhtpool = xht_cm.__enter__()
            sc_cm = tc.tile_pool(name="scpool", bufs=1)
            scpool = sc_cm.__enter__()
            g_cm = tc.tile_pool(name="gpool", bufs=1)
            gpool = g_cm.__enter__()
            xio_cm = tc.tile_pool(name="xio", bufs=1)
            xio = xio_cm.__enter__()

            # ---- x hi/lo DMAs issued FIRST so phase G starts early ----
            xh_s = xio.tile([128, NCH, C], bf16, name="xh_s")
            xl2_s = xio.tile([128, NCH, C], bf16, name="xl2_s")
            for g in range(8):
                eng_a = nc.sync if g % 2 == 0 else nc.scalar
                eng_b = nc.scalar if g % 2 == 0 else nc.sync
                eng_a.dma_start(
                    xh_s[:, 2 * g : 2 * (g + 1), :],
                    xh_d[:, 2 * g * C : 2 * (g + 1) * C],
                )
                eng_b.dma_start(
                    xl2_s[:, 2 * g : 2 * (g + 1), :],
                    xl2_d[:, 2 * g * C : 2 * (g + 1) * C],
                )
            xht_s = xhtpool.tile([128, 6, ROWS], bf16, name="xht_s")
            nc.scalar.dma_start(xht_s[:], xht_d.ap())

            # ---- weights / constants (whole-kernel) ----
            wvt_s = wpool.tile([128, 6, C], bf16)
            nc.sync.dma_start(wvt_s[:], wvt_d.ap())
            wpt_s = wpool.tile([128, 6, C], bf16)
            nc.sync.dma_start(wpt_s[:], wpt_d.ap())
            bias_s = wpool.tile([128, C], f32)
            nc.sync.dma_start(bias_s[:], bias_d[:])
            tvec_s = wpool.tile([128, 4], f32)
            nc.sync.dma_start(tvec_s[:], tvec_d[:])
            identb = wpool.tile([128, 128], bf16)
            nc.sync.dma_start(identb[:], identb_d[:])
            identf = wpool.tile([128, 128], f32)
            nc.sync.dma_start(identf[:], identf_d[:])
            wqth_s = wpool.tile([128, 6, C], bf16, name="wqth_s")
            nc.scalar.dma_start(wqth_s[:], wqth_d.ap())
            wqtl_s = wpool.tile([128, 6, C], bf16, name="wqtl_s")
            nc.scalar.dma_start(wqtl_s[:], wqtl_d.ap())

            # DRAM staging
            ssend = drpool.tile([4, HD, HD], f32)
            srecv = drpool.tile([4, HD, HD], f32)
            a2asend = [
                drpool.tile([8, 4, ECH, ROWS], bf16, name=f"a2as{g}")
                for g in range(NCHUNK)
            ]
            a2arecv = [
                drpool.tile([8, 4, ECH, ROWS], bf16, name=f"a2ar{g}")
                for g in range(NCHUNK)
            ]
            vband = [
                drpool.tile([2, 4 * ECH, N], bf16, name=f"vband{g}")
                for g in range(NCHUNK)
            ]

            # block-diag softmax matrix, zeroed early on idle gpsimd
            sm_s = smpool.tile([128, 6, C], bf16, name="sm_s")
            nc.gpsimd.memset(sm_s[:], 0)

            # g-pool: g_f (fp32 G), g_bh/g_bl (bf16 hi/lo split)
            g_f = gpool.tile([128, 6, C], f32, name="g_f")
            g_bh = gpool.tile([128, 6, C], bf16, name="g_bh")
            g_bl = gpool.tile([128, 6, C], bf16, name="g_bl")

            msb_cm = tc.tile_pool(name="msbpool", bufs=1)
            msbpool = msb_cm.__enter__()
            msb = msbpool.tile([128, 6, C], f32, name="msb")

            # ============ Phase G: M = 1/2 xh^T xh + xh^T xl2 ==============
            with tc.tile_pool(name="ps_g", bufs=1, space="PSUM") as ps_g:
                for half in range(2):
                    csl = slice(384 * half, 384 * (half + 1))
                    mps = [
                        ps_g.tile([128, 384], f32, tag=f"m{j}", name=f"mps{j}")
                        for j in range(6)
                    ]
                    for ch in range(NCH):
                        for j in range(6):
                            lhs = xh_s[:, ch, 128 * j : 128 * (j + 1)]
                            nc.tensor.matmul(
                                mps[j][:], lhs, xh_s[:, ch, csl],
                                start=(ch == 0), stop=False,
                                skip_group_check=True,
                            )
                            nc.tensor.matmul(
                                mps[j][:], lhs, xl2_s[:, ch, csl],
                                start=False, stop=(ch == NCH - 1),
                                skip_group_check=True,
                            )
                    for j in range(6):
                        if j % 2 == 0:
                            nc.vector.tensor_scalar_mul(
                                msb[:, j, csl], mps[j][:], 0.5
                            )
                        else:
                            nc.scalar.mul(msb[:, j, csl], mps[j][:], 0.5)

            # ---- G = M + M^T, then bf16 hi/lo split of G ----
            with tc.tile_pool(name="ps_sym", bufs=2, space="PSUM") as ps_sym:
                for i in range(6):
                    trow = ps_sym.tile([128, C], f32, tag="trow")
                    for j in range(6):
                        nc.tensor.matmul(
                            trow[:, 128 * j : 128 * (j + 1)],
                            msb[:, j, 128 * i : 128 * (i + 1)],
                            identf[:], is_transpose=True,
                            start=True, stop=True, skip_group_check=True,
                        )
                    nc.vector.tensor_add(g_f[:, i, :], msb[:, i, :], trow[:])
                    nc.scalar.copy(g_bh[:, i, :], g_f[:, i, :])
                    nc.vector.tensor_sub(g_bl[:, i, :], g_f[:, i, :], g_bh[:, i, :])

            msb_cm.__exit__(None, None, None)  # free msb (18KB)
            xio_cm.__exit__(None, None, None)  # free xh/xl2 (49KB)

            # ============ Phase A: A = G Wq^T, 3 bf16 hi/lo passes =========
            wka_cm = tc.tile_pool(name="wkapool", bufs=1)
            wkapool = wka_cm.__enter__()
            wkt_s = wkapool.tile([128, 6, C], f32, name="wkt_s")
            nc.scalar.dma_start(wkt_s[:], wkt_d.ap())
            a_s = wkapool.tile([128, 6, C], f32, name="a_s")
            APASS = [("hh", None, None), ("hl", None, None), ("lh", None, None)]
            with tc.tile_pool(name="ps_a", bufs=1, space="PSUM") as ps_a:
                for half in range(2):
                    qsl = slice(384 * half, 384 * (half + 1))
                    for i in range(6):
                        ap_t = ps_a.tile(
                            [128, 384], f32, tag=f"a{i}", name=f"aps{i}"
                        )
                        nmm = 0
                        for gt, wt in ((g_bh, wqth_s), (g_bh, wqtl_s), (g_bl, wqth_s)):
                            for j in range(6):
                                nmm += 1
                                nc.tensor.matmul(
                                    ap_t[:],
                                    gt[:, j, 128 * i : 128 * (i + 1)],
                                    wt[:, j, qsl],
                                    start=(nmm == 1),
                                    stop=(nmm == 18),
                                    skip_group_check=True,
                                )
                        if i % 2 == 0:
                            nc.vector.tensor_copy(a_s[:, i, qsl], ap_t[:])
                        else:
                            nc.scalar.copy(a_s[:, i, qsl], ap_t[:])

            # ---- scores S^T_h = wk-contract A (fp32), then AllReduce ----
            sp_lo = scpool.tile([128, 4, HD], f32)
            sp_hi = scpool.tile([64, 4, HD], f32)
            with tc.tile_pool(name="ps_sc", bufs=2, space="PSUM") as ps_sc:
                for h in range(4):
                    hsl = slice(HD * h, HD * (h + 1))
                    st_lo = ps_sc.tile([128, HD], f32, tag="stlo", name="st_lo")
                    st_hi = ps_sc.tile([64, HD], f32, tag="sthi", name="st_hi")
                    for i in range(6):
                        nc.tensor.matmul(
                            st_lo[:],
                            wkt_s[:, i, HD * h : HD * h + 128],
                            a_s[:, i, hsl],
                            start=(i == 0), stop=(i == 5),
                            skip_group_check=True,
                        )
                    for i in range(6):
                        nc.tensor.matmul(
                            st_hi[:],
                            wkt_s[:, i, HD * h + 128 : HD * (h + 1)],
                            a_s[:, i, hsl],
                            start=(i == 0), stop=(i == 5),
                            skip_group_check=True,
                        )
                    nc.vector.tensor_copy(sp_lo[:, h, :], st_lo[:])
                    nc.scalar.copy(sp_hi[:, h, :], st_hi[:])
            for h in range(4):
                nc.sync.dma_start(ssend[h, 0:128, :], sp_lo[:, h, :])
                nc.sync.dma_start(ssend[h, 128:HD, :], sp_hi[:, h, :])
            nc.gpsimd.collective_compute(
                "AllReduce",
                mybir.AluOpType.add,
                replica_groups=RG,
                ins=[ssend.opt()],
                outs=[srecv.opt()],
            )
            wka_cm.__exit__(None, None, None)  # free wkt_s + a_s (36KB)
            g_cm.__exit__(None, None, None)  # free g tiles (36KB)

            # ============ Phase V (packed, covers score-AllReduce) ==========
            vt_cm = tc.tile_pool(name="vtpool", bufs=1)
            vtpool = vt_cm.__enter__()
            vt_s = vtpool.tile([128, 6, ROWS], bf16, name="vt_s")
            with tc.tile_pool(name="ps_v", bufs=1, space="PSUM") as ps_v:
                for w in range(NW):
                    nsl = slice(512 * w, 512 * (w + 1))
                    for k in range(6):
                        vp = ps_v.tile(
                            [128, 512], f32, tag=f"v{k}", name=f"vps{k}"
                        )
                        for cb in range(6):
                            nc.tensor.matmul(
                                vp[:],
                                wvt_s[:, cb, 128 * k : 128 * (k + 1)],
                                xht_s[:, cb, nsl],
                                start=(cb == 0), stop=(cb == 5),
                            )
                        if k % 2 == 0:
                            nc.vector.tensor_copy(vt_s[:, k, nsl], vp[:])
                        else:
                            nc.scalar.copy(vt_s[:, k, nsl], vp[:])

            # ---- softmax per head from reduced scores -> block-diag SM ----
            sr_lo, sr_hi = sp_lo, sp_hi
            for h in range(4):
                nc.sync.dma_start(sr_lo[:, h, :], srecv[h, 0:128, :])
                nc.sync.dma_start(sr_hi[:, h, :], srecv[h, 128:HD, :])
            for h in range(4):
                smt = {}
                for src_t, nrow in ((sr_lo, 128), (sr_hi, 64)):
                    ap_in = src_t[0:nrow, h, :]
                    mx = scpool.tile([nrow, 1], f32, tag=f"mx{nrow}", name="mx")
                    nc.vector.tensor_reduce(
                        mx[:], ap_in,
                        axis=mybir.AxisListType.X, op=mybir.AluOpType.max,
                    )
                    nmt = scpool.tile([nrow, 1], f32, tag=f"nm{nrow}", name="nmt")
                    nc.vector.tensor_mul(nmt[:], mx[:], tvec_s[:nrow, h : h + 1])
                    nc.vector.tensor_scalar_mul(nmt[:], nmt[:], -1.0)
                    p_t = scpool.tile([nrow, HD], f32, tag=f"p{nrow}", name="p_t")
                    ssum = scpool.tile([nrow, 1], f32, tag=f"s{nrow}", name="ssum")
                    nc.scalar.activation(
                        p_t[:], ap_in,
                        mybir.ActivationFunctionType.Exp,
                        bias=nmt[:], scale=tvec_s[:nrow, h : h + 1],
                        accum_out=ssum[:],
                    )
                    rcp = scpool.tile([nrow, 1], f32, tag=f"r{nrow}", name="rcp")
                    nc.vector.reciprocal(rcp[:], ssum[:])
                    smt_t = scpool.tile(
                        [nrow, HD], f32, tag=f"smt{nrow}", name="smt_t"
                    )
                    nc.vector.tensor_scalar_mul(smt_t[:], p_t[:], rcp[:])
                    smt[nrow] = smt_t
                with tc.tile_pool(name=f"ps_smt{h}", bufs=1, space="PSUM") as pst:
                    tlo = pst.tile([128, HD], f32, name="tlo")
                    thi = pst.tile([64, HD], f32, name="thi")
                    nc.tensor.matmul(
                        tlo[:, 0:128], smt[128][:, 0:128], identf[:],
                        is_transpose=True, start=True, stop=True,
                        skip_group_check=True,
                    )
                    nc.tensor.matmul(
                        tlo[:, 128:HD], smt[64][:, 0:128], identf[:64, 0:64],
                        is_transpose=True, start=True, stop=True,
                        skip_group_check=True,
                    )
                    nc.tensor.matmul(
                        thi[:, 0:128], smt[128][:, 128:HD], identf[:],
                        is_transpose=True, start=True, stop=True,
                        skip_group_check=True,
                    )
                    nc.tensor.matmul(
                        thi[:, 128:HD], smt[64][:, 128:HD], identf[:64, 0:64],
                        is_transpose=True, start=True, stop=True,
                        skip_group_check=True,
                    )
                    # scatter into packed block-diag rows 192h + a
                    csl = slice(HD * h, HD * (h + 1))
                    for (src, a0, nr) in ((tlo, 0, 128), (thi, 128, 64)):
                        r0 = HD * h + a0
                        placed = 0
                        while placed < nr:
                            j = (r0 + placed) // 128
                            p0 = (r0 + placed) % 128
                            cnt = min(128 - p0, nr - placed)
                            nc.vector.tensor_copy(
                                sm_s[p0 : p0 + cnt, j, csl],
                                src[placed : placed + cnt, :],
                            )
                            placed += cnt

            # ============ Phase X: x_caT = SM^T-contract vT (packed) ========
            XJ = {0: (0, 1), 1: (0, 1, 2), 2: (1, 2), 3: (3, 4), 4: (3, 4, 5), 5: (4, 5)}
            xc_cm = tc.tile_pool(name="xcpool", bufs=1)
            xcpool = xc_cm.__enter__()
            xcat_s = xcpool.tile([128, 6, ROWS], bf16, name="xcat_s")
            # strip pieces grouped by the xcat tile j they read, so each
            # tile's a2a send staging fires as soon as that tile is done
            strip_by_tile = {k: [] for k in range(6)}
            for g in range(NCHUNK):
                for i in range(8):
                    for h in range(4):
                        r0 = 192 * h + 24 * i + ECH * g
                        placed = 0
                        while placed < ECH:
                            j = (r0 + placed) // 128
                            p0 = (r0 + placed) % 128
                            cnt = min(128 - p0, ECH - placed)
                            strip_by_tile[j].append((g, i, h, placed, p0, cnt))
                            placed += cnt
            with tc.tile_pool(name="ps_x", bufs=1, space="PSUM") as ps_x:
                for k in range(6):
                    for w in range(NW):
                        nsl = slice(512 * w, 512 * (w + 1))
                        xp = ps_x.tile(
                            [128, 512], f32, tag=f"x{w}", name=f"xps{w}"
                        )
                        js = XJ[k]
                        for idx, j in enumerate(js):
                            nc.tensor.matmul(
                                xp[:],
                                sm_s[:, j, 128 * k : 128 * (k + 1)],
                                vt_s[:, j, nsl],
                                start=(idx == 0),
                                stop=(idx == len(js) - 1),
                            )
                        if w % 2 == 0:
                            nc.vector.tensor_copy(xcat_s[:, k, nsl], xp[:])
                        else:
                            nc.scalar.copy(xcat_s[:, k, nsl], xp[:])
                    for (g, i, h, placed, p0, cnt) in strip_by_tile[k]:
                        if g == 0:
                            nc.sync.dma_start(
                                a2asend[g][i, h, placed : placed + cnt, :],
                                xcat_s[p0 : p0 + cnt, k, :],
                            )
                if k == 5:
                    for kk in range(6):
                        for (g, i, h, placed, p0, cnt) in strip_by_tile[kk]:
                            if g > 0:
                                nc.sync.dma_start(
                                    a2asend[g][i, h, placed : placed + cnt, :],
                                    xcat_s[p0 : p0 + cnt, kk, :],
                                )

            # ============ Tail: chunked a2a + projection pipeline ===========
            # chunk g: e-cols [24i+6g, 24i+6g+6) per dst i, all heads
            with (
                tc.tile_pool(name="zpool", bufs=3) as zpool,
                tc.tile_pool(name="opool", bufs=2) as opool,
                tc.tile_pool(name="ps_zt", bufs=2, space="PSUM") as ps_zt,
                tc.tile_pool(name="ps_o", bufs=2, space="PSUM") as ps_o,
            ):
                for g in range(NCHUNK):
                    nc.gpsimd.collective_compute(
                        "AllToAll",
                        mybir.AluOpType.bypass,
                        replica_groups=[list(range(8))],
                        ins=[a2asend[g].opt()],
                        outs=[a2arecv[g].opt()],
                    )
                    # assemble vband chunk: rows 4e+h, n from 4 senders
                    for b in range(2):
                        vb_v = vband[g][b].rearrange("(e h) n -> h e n", h=4)
                        for j in range(4):
                            eng = nc.sync if b == 0 else nc.scalar
                            eng.dma_start(
                                vb_v[:, :, ROWS * j : ROWS * (j + 1)],
                                a2arecv[g][4 * b + j],
                            )
                    # projection tiles: 2 per batch per chunk
                    for b in range(2):
                        for tt in range(ECH // 3):
                            t = (ECH // 3) * g + tt
                            z_nat = zpool.tile([128, C], bf16, tag="zn")
                            zeng = nc.scalar if b == 0 else nc.sync
                            zeng.dma_start(
                                z_nat[:],
                                vband[g][b, 12 * tt : 12 * (tt + 1), :],
                            )
                            ztp = ps_zt.tile([128, C], bf16, tag="ztp")
                            for j in range(6):
                                nc.tensor.matmul(
                                    ztp[:, 128 * j : 128 * (j + 1)],
                                    z_nat[:, 128 * j : 128 * (j + 1)],
                                    identb[:],
                                    is_transpose=True,
                                    start=True, stop=True,
                                    skip_group_check=True,
                                )
                            zt = zpool.tile([128, 6, 128], bf16, tag="zt")
                            if (b + tt) % 2 == 0:
                                nc.vector.tensor_copy(
                                    zt[:],
                                    ztp[:].rearrange("p (o f) -> p o f", f=128),
                                )
                            else:
                                nc.scalar.copy(
                                    zt[:],
                                    ztp[:].rearrange("p (o f) -> p o f", f=128),
                                )
                            o1 = ps_o.tile([128, 384], f32, tag="o1")
                            o2 = ps_o.tile([128, 384], f32, tag="o2")
                            for j in range(6):
                                nc.tensor.matmul(
                                    o1[:], zt[:, j, :], wpt_s[:, j, 0:384],
                                    start=(j == 0), stop=(j == 5),
                                )
                            for j in range(6):
                                nc.tensor.matmul(
                                    o2[:], zt[:, j, :], wpt_s[:, j, 384:C],
                                    start=(j == 0), stop=(j == 5),
                                )
                            out_sb = opool.tile([128, C], bf16, tag="ob")
                            nc.vector.tensor_add(
                                out_sb[:, 0:384], o1[:], bias_s[:, 0:384]
                            )
                            nc.vector.tensor_add(
                                out_sb[:, 384:C], o2[:], bias_s[:, 384:C]
                            )
                            oeng = nc.sync if b == 0 else nc.scalar
                            oeng.dma_start(
                                out_d[b, 128 * t : 128 * (t + 1), :], out_sb[:]
                            )

            xc_cm.__exit__(None, None, None)
            vt_cm.__exit__(None, None, None)
            sc_cm.__exit__(None, None, None)
            xht_cm.__exit__(None, None, None)
            sm_cm.__exit__(None, None, None)

    nc.compile()
    return nc


def _get_nc():
    if "nc" not in _cached:
        _cached["nc"] = _build()
    return _cached["nc"]


def _pretile(a, p=128):
    # [o*p, f] -> [p, o*f], matching SBUF tile [p, o, f]
    rows, f = a.shape
    o = rows // p
    return np.ascontiguousarray(
        a.reshape(o, p, f).transpose(1, 0, 2).reshape(p, o * f)
    )


def _prep_in_maps(x, w_qkv, temperature, w_proj, b_proj):
    import ml_dtypes

    bf = ml_dtypes.bfloat16
    x = np.ascontiguousarray(np.asarray(x, dtype=np.float32))
    w_qkv = np.asarray(w_qkv, dtype=np.float32)
    temperature = np.asarray(temperature, dtype=np.float32)
    w_proj = np.asarray(w_proj, dtype=np.float32)
    b_proj = np.asarray(b_proj, dtype=np.float32)

    wqt = np.ascontiguousarray(w_qkv[0:C].T)
    wqth = wqt.astype(bf)
    wqtl = (wqt - wqth.astype(np.float32)).astype(bf)
    wqth = _pretile(wqth)
    wqtl = _pretile(wqtl)
    wkt = _pretile(np.ascontiguousarray(w_qkv[C : 2 * C].T))
    wvt = _pretile(np.ascontiguousarray(w_qkv[2 * C : 3 * C].T).astype(bf))
    wpt = _pretile(np.ascontiguousarray(w_proj.T).astype(bf))

    bias = np.ascontiguousarray(np.broadcast_to(b_proj, (128, C)))
    tvec = np.broadcast_to(
        temperature.reshape(1, H).astype(np.float32), (128, H)
    ).copy()
    identb = np.eye(128, dtype=np.float32).astype(bf)
    identf = np.eye(128, dtype=np.float32)

    in_maps = []
    for c in range(NCORE):
        b, r = c // 4, c % 4
        xs = x[b, ROWS * r : ROWS * (r + 1), :]
        xh = xs.astype(bf)
        xl2 = ((xs - xh.astype(np.float32)) * 2.0).astype(bf)
        in_maps.append(
            {
                "xh": _pretile(np.ascontiguousarray(xh)),
                "xl2": _pretile(np.ascontiguousarray(xl2)),
                "xht": _pretile(np.ascontiguousarray(xs.T).astype(bf)),
                "wqth": wqth,
                "wqtl": wqtl,
                "wkt": wkt,
                "wvt": wvt,
                "wpt": wpt,
                "bias": bias,
                "tvec": tvec,
                "identb": identb,
                "identf": identf,
            }
        )
    return in_maps


def kernel(x, w_qkv, temperature, w_proj, b_proj):
    from concourse.bass_utils import run_bass_kernel_spmd

    nc = _get_nc()
    in_maps = _prep_in_maps(x, w_qkv, temperature, w_proj, b_proj)
    res = run_bass_kernel_spmd(nc, in_maps, core_ids=list(range(NCORE)))
    out = np.empty((B, N, C), np.float32)
    for c in range(NCORE):
        o = res.results[c]["out"]  # [2, 1024, C] bf16
        for b in range(B):
            out[b, 1024 * c : 1024 * (c + 1), :] = o[b].astype(np.float32)
    return out
